# revision 4
# baseline (speedup 1.0000x reference)
"""RWKV block (T=8192, D=2048) on 8 Trainium2 NeuronCores — v3.

v2 (phase-major 2x544-token chunks, fp16 GEMM operands, 64-token warmup)
+ latency engineering from the timeline model: LN applies run all-fp16 in
SBUF (2x DVE rate) with mean/rstd converted once per sub-block on the Act
engine; stats PSUM double-buffered so sb0/sb1 overlap; mixes and GEMM
guards are per-sub-block so each GEMM starts after half the apply work;
chunk ch+1's x-DMA + LN1 + mixk are emitted inside ch's FFN phases to
erase the chunk-boundary stall; constants arrive in one packed DMA.
"""
import sys
if '/opt/trn_rl_repo' not in sys.path:
    sys.path.insert(0, '/opt/trn_rl_repo')

from contextlib import ExitStack
import numpy as np

import concourse.bass as bass
import concourse.tile as tile
from concourse import bacc, mybir
from concourse.bass import _add_dep_helper
from concourse.bass_utils import run_bass_kernel_spmd

F32 = mybir.dt.float32
F32R = mybir.dt.float32r
F16 = mybir.dt.float16
AF = mybir.ActivationFunctionType
OP = mybir.AluOpType

D = 2048
T = 8192
NCORES = 8
TLOC = T // NCORES          # 1024 main tokens per core
H = 64                      # warmup tokens
TBUF = H + TLOC             # 1088
NCH = 2                     # chunks per core
C = TBUF // NCH             # 544 tokens per chunk
BS = C // 2                 # 272-token GEMM sub-block (PSUM bank = 512 f32)
NT = D // 128               # 16 partition tiles
JQ = 2                      # j-tiles per weight panel group (256 out chans)
NJG = NT // JQ              # 8 panel groups per GEMM

WNAMES = ['wk', 'wv', 'wr', 'wo', 'wfk', 'wfv', 'wfr']
BNAMES = ['bk', 'bv', 'br', 'bo', 'bfk', 'bfv', 'bfr']
VNAMES = ['mixk', 'mixv', 'mixr', 'fmixk', 'fmixr', 'ew', 'eu', 'cmask']
CVNAMES = BNAMES + VNAMES


def _prefer_combined_act_table():
    """Steer the act-table chooser so Ln and Exp both resolve to
    natural_log_exp_and_others (one table for the rstd chain AND the EK
    exponentials) instead of ping-ponging LoadActFuncSet (1.28us each).
    Set order/names/indices are preserved — walrus still sees the original
    act_info.json ids — we only hide exp/ln from the smaller sets so the
    first set that satisfies them is the combined one."""
    import functools
    import concourse.hw_specs as hw_specs
    if getattr(bacc, '_act_tables_tuned', False):
        return
    orig = hw_specs.get_activation_tables

    @functools.cache
    def tuned(arch):
        t = {k: set(v) for k, v in orig(arch).items()}
        for name, s in t.items():
            if name == 'natural_log_exp_and_others':
                continue
            s.discard(mybir.ActivationFunctionType.Exp)
            s.discard(mybir.ActivationFunctionType.Ln)
        return t

    bacc.get_activation_tables = tuned
    bacc._act_tables_tuned = True


def build_kernel():
    _prefer_combined_act_table()
    nc = bacc.Bacc()
    xP = nc.declare_dram_parameter('xP', [2 * NCH, 128, NT * BS], F16,
                                   isOutput=False)
    cvP = nc.declare_dram_parameter('cvP', [128, len(CVNAMES) * NT], F32,
                                    isOutput=False)
    onescol = nc.declare_dram_parameter('onescol', [128, 1], F16, isOutput=False)
    onesrow = nc.declare_dram_parameter('onesrow', [1, 128], F32R, isOutput=False)
    wd = {n: nc.declare_dram_parameter(n, [NJG, 128, NT * JQ * 128], F16,
                                       isOutput=False)
          for n in WNAMES}
    outT = nc.declare_dram_parameter('outT', [D, TBUF], F16, isOutput=True)
    outTt = outT.rearrange('(n p) t -> n p t', p=128)

    with tile.TileContext(nc) as tc:
        with ExitStack() as ctx:
            kern(ctx, tc, xP, wd, cvP, outTt, onescol, onesrow)
    nc.compile()
    return nc


def kern(ctx, tc, xP, wd, cvP, outTt, onescol, onesrow):
    nc = tc.nc

    cons = ctx.enter_context(tc.tile_pool(name='cons', bufs=1))
    colp = ctx.enter_context(tc.tile_pool(name='colp', bufs=1))
    wpool = ctx.enter_context(tc.tile_pool(name='wpool', bufs=2))
    ap = ctx.enter_context(tc.tile_pool(name='ap', bufs=1))
    mixp = ctx.enter_context(tc.tile_pool(name='mixp', bufs=4))
    xop = ctx.enter_context(tc.tile_pool(name='xop', bufs=2))
    wkvp = ctx.enter_context(tc.tile_pool(name='wkvp', bufs=2))
    scr = ctx.enter_context(tc.tile_pool(name='scr', bufs=2))
    rows = ctx.enter_context(tc.tile_pool(name='rows', bufs=2))
    psg = ctx.enter_context(tc.tile_pool(name='psg', bufs=4, space='PSUM'))
    pss = ctx.enter_context(tc.tile_pool(name='pss', bufs=2, space='PSUM'))

    # ones first (tiny), then x for chunk 0; constants behind (packed DMA)
    xall_c = {}

    def load_x(ch):
        xts = [xop.tile([128, NT * BS], F16, tag='xall', name=f'xall{ch}{sb}')
               for sb in range(2)]
        for sb in range(2):
            nc.sync.dma_start(xts[sb][:], xP[ch * 2 + sb])
        xall_c[ch] = [[xts[sb][:, i * BS:(i + 1) * BS] for i in range(NT)]
                      for sb in range(2)]

    ones = cons.tile([128, 1], F16, tag='ones', name='ones')
    nc.sync.dma_start(ones[:], onescol[:])
    load_x(0)
    cvt = cons.tile([128, len(CVNAMES) * NT], F32, tag='cvt', name='cvt')
    nc.sync.dma_start(cvt[:], cvP[:])
    cv = {}
    for ni, n in enumerate(CVNAMES):
        cv[n] = [cvt[:, ni * NT + i:ni * NT + i + 1] for i in range(NT)]
    ones_row = cons.tile([1, 128], F32R, tag='ones_row', name='ones_row')
    nc.sync.dma_start(ones_row[:], onesrow[:])
    # trigger the (single) act-table load under the x DMA
    lnp0 = cons.tile([128, 1], F32, tag='lnp0', name='lnp0')
    nc.scalar.activation(lnp0[:], ones[:], AF.Ln)

    def pe_guard(aps):
        """Single-wait carrier for fused-LDW matmul chains (see v2)."""
        eng = nc.tensor
        inst = mybir.InstNoOp(
            name=nc.get_next_instruction_name(),
            text_hint='pe_guard', bass_nofuse=True,
            ins=[eng.lower_ap(a) for a in aps])
        return eng.add_instruction(inst)

    FULL = [(0, BS), (BS, BS)]

    def gemm(wname, rhs_sb, out_cb, ranges=None, split_first=False,
             sbs=(0, 1), wtag='wt'):
        """out[j, sb] = sum_kt w[kt, j].T @ rhs_sb[sb][kt] over ranges[sb].
        Guards are per (j0, sb) so sb0 chains start before sb1 rhs exists;
        with split_first, the first chain's guards are interleaved per
        4-tile group so the PE paces along the producing DVE chain.
        sbs selects which sub-blocks to emit (for sb-outer phases)."""
        if ranges is None:
            ranges = FULL
        for j0 in range(NJG):
            wt = wpool.tile([128, NT * JQ * 128], F16, tag=wtag,
                            name=f'wt_{wname}{j0}')
            nc.sync.dma_start(wt[:], wd[wname][j0])
            for sb in sbs:
                off, w = ranges[sb]
                split = split_first and j0 == 0 and sb == sbs[0]
                guards = [None] * 4
                if not split:
                    guards = [pe_guard([wt[:]] + [rhs_sb[sb][kt]
                                                  for kt in range(NT)])] * 4
                for jj in range(JQ):
                    pt = psg.tile([128, w], F32, tag='ps', name='ps')
                    for kt in range(NT):
                        if split and jj == 0 and kt % 4 == 0:
                            guards[kt // 4] = pe_guard(
                                [wt[:]] + [rhs_sb[sb][k2]
                                           for k2 in range(kt, kt + 4)])
                        lo = kt * JQ * 128 + jj * 128
                        mm = nc.tensor.matmul(
                            pt[:], wt[:, lo:lo + 128], rhs_sb[sb][kt],
                            start=(kt == 0), stop=(kt == NT - 1))
                        _add_dep_helper(mm.ins, guards[kt // 4].ins,
                                        sync=False, reason='order after guard')
                    out_cb(j0 * JQ + jj, sb, pt[:], off, w)

    def ln_stats(xs):
        """Per-token mean/rstd via ones-matmuls; returns fp16 SBUF
        broadcast tiles (s16, m16) [128, BS]."""
        ps_s = pss.tile([1, BS], F32, tag='st0', name='st0')
        ps_q = pss.tile([1, BS], F32, tag='st1', name='st1')
        sq0 = scr.tile([128, BS], F16, tag='sq', name='sq', bufs=4)
        nc.scalar.activation(sq0[:], xs[0], AF.Square)
        guard = pe_guard(list(xs) + [sq0[:], ones[:]])
        for kt in range(NT):
            if kt == 0:
                sq = sq0
            else:
                sq = scr.tile([128, BS], F16, tag='sq', name='sq', bufs=4)
                nc.scalar.activation(sq[:], xs[kt], AF.Square)
            mm = nc.tensor.matmul(ps_s[:], ones[:], xs[kt],
                                  start=(kt == 0), stop=(kt == NT - 1))
            _add_dep_helper(mm.ins, guard.ins, sync=False, reason='g')
            mm2 = nc.tensor.matmul(ps_q[:], ones[:], sq[:],
                                   start=(kt == 0), stop=(kt == NT - 1))
            _add_dep_helper(mm2.ins, guard.ins, sync=False, reason='g')
        mean = rows.tile([1, BS], F32R, tag='mean', name='mean')
        var = rows.tile([1, BS], F32, tag='var', name='var')
        m2 = rows.tile([1, BS], F32, tag='m2', name='m2')
        nc.vector.tensor_scalar_mul(mean[:], ps_s[:], 1.0 / D)
        nc.vector.tensor_scalar_mul(var[:], ps_q[:], 1.0 / D)
        nc.vector.tensor_mul(m2[:], mean[:], mean[:])
        nc.vector.tensor_sub(var[:], var[:], m2[:])
        nc.vector.tensor_scalar_add(var[:], var[:], 1e-5)
        lnv = rows.tile([1, BS], F32, tag='lnv', name='lnv')
        nc.scalar.activation(lnv[:], var[:], AF.Ln)
        rstd = rows.tile([1, BS], F32R, tag='rstd', name='rstd')
        nc.scalar.activation(rstd[:], lnv[:], AF.Exp, scale=-0.5)
        s_b = pss.tile([128, BS], F32, tag='st0', name='s_b')
        m_b = pss.tile([128, BS], F32, tag='st1', name='m_b')
        guard2 = pe_guard([rstd[:], mean[:], ones_row[:]])
        mmb = nc.tensor.matmul(s_b[:], ones_row[:], rstd[:], start=True, stop=True)
        _add_dep_helper(mmb.ins, guard2.ins, sync=False, reason='g2')
        mmb2 = nc.tensor.matmul(m_b[:], ones_row[:], mean[:], start=True, stop=True)
        _add_dep_helper(mmb2.ins, guard2.ins, sync=False, reason='g2')
        # PSUM f32 -> SBUF fp16 once (Act engine), so applies run 2x on DVE
        s16 = scr.tile([128, BS], F16, tag='s16', name='s16')
        m16 = scr.tile([128, BS], F16, tag='m16', name='m16')
        nc.scalar.activation(s16[:], s_b[:], AF.Copy)
        nc.scalar.activation(m16[:], m_b[:], AF.Copy)
        return s16, m16

    def mix_one(Ub, mixname, tagp, i, off, w):
        """d = U[t]-U[t-1], mt = U[t-1] + mix*d over cols [off, off+w)."""
        d = scr.tile([128, w], F16, tag='d1', name=f'd{tagp}', bufs=4)
        nc.vector.tensor_sub(d[:], Ub[i][:, 1 + off:1 + off + w],
                             Ub[i][:, off:off + w])
        mt = mixp.tile([128, w], F16, tag=f'mix{i}', name=f'{tagp}{i}')
        nc.vector.scalar_tensor_tensor(
            mt[:], d[:], cv[mixname][i],
            Ub[i][:, off:off + w], OP.mult, OP.add)
        return mt[:]

    def ln_sb(xs_i, Ub, sb, mix=None, mixes=None):
        """One sub-block of a LayerNorm: stats + all-fp16 applies, with
        per-tile lerps fused right behind each apply (mix or mixes)."""
        if mixes is None:
            mixes = [mix] if mix is not None else []
        mts = [[None] * NT for _ in mixes]
        s16, m16 = ln_stats(xs_i)
        for i in range(NT):
            t1 = scr.tile([128, BS], F16, tag='ut', name='ut', bufs=4)
            nc.vector.tensor_sub(t1[:], xs_i[i], m16[:])
            nc.vector.tensor_mul(Ub[i][:, 1 + sb * BS:1 + (sb + 1) * BS],
                                 t1[:], s16[:])
            for mi, (mixname, tagp, ranges) in enumerate(mixes):
                off, w = ranges[sb]
                mts[mi][i] = mix_one(Ub, mixname, tagp, i, off, w)
        return mts if len(mts) != 1 else mts[0]

    def ln_to(xs_sb, Ub, UcolT, mix=None):
        """Full LayerNorm into U buffer [128, C+1] (lead col from UcolT)."""
        for i in range(NT):
            nc.vector.tensor_copy(Ub[i][:, 0:1], UcolT[i][:])
        mts = [ln_sb(xs_sb[sb], Ub, sb, mix=mix) for sb in range(2)]
        for i in range(NT):
            nc.vector.tensor_copy(UcolT[i][:], Ub[i][:, C:C + 1])
        return mts

    def mk_mix(Ub, mixname, tagp, ranges=None):
        """Per-sb lerp tiles for a whole phase (non-latency-critical)."""
        if ranges is None:
            ranges = FULL
        return [[mix_one(Ub, mixname, tagp, i, off, w) for i in range(NT)]
                for sb, (off, w) in enumerate(ranges)]

    # persistent cross-chunk state
    Ucol = [colp.tile([128, 1], F16, tag=f'uc{i}', name=f'uc{i}')
            for i in range(NT)]
    U2col = [colp.tile([128, 1], F16, tag=f'u2c{i}', name=f'u2c{i}')
             for i in range(NT)]
    Acol = [colp.tile([128, 1], F16, tag=f'acl{i}', name=f'acl{i}')
            for i in range(NT)]
    Bcol = [colp.tile([128, 1], F16, tag=f'bcl{i}', name=f'bcl{i}')
            for i in range(NT)]
    for i in range(NT):
        nc.vector.memset(Ucol[i][:], 0.0)
        nc.vector.memset(U2col[i][:], 0.0)
        nc.vector.memset(Acol[i][:], 0.0)
        nc.vector.memset(Bcol[i][:], 0.0)

    def att_front(ch):
        """x-DMA + LN1 + fused mixk for chunk ch (emitted early for ch>0)."""
        if ch not in xall_c:
            load_x(ch)
        U = [ap.tile([128, C + 1], F16, tag=f'u{i}', name=f'u{i}_{ch}')
             for i in range(NT)]
        ink = ln_to(xall_c[ch], U, Ucol, mix=('mixk', 'mk', FULL))
        return U, ink

    front = att_front(0)

    for ch in range(NCH):
        U, ink = front
        xs_sb = xall_c[ch]
        # warmup cols (chunk 0, sb 0 only) are needed by k/v (scan history)
        # and by rz/U2 at the last warmup col; r/atto skip cols < H-1 and
        # the FFN skips cols < H.
        RO = [(H - 1, BS - H + 1), (BS, BS)] if ch == 0 else FULL
        RF = [(H, BS - H), (BS, BS)] if ch == 0 else FULL

        # ---- k GEMM -> EK = exp(k) ----
        EK = [ap.tile([128, C], F16, tag=f'ek{i}', name=f'ek{i}_{ch}')
              for i in range(NT)]
        k_cb = (lambda j, sb, ps, off, w: nc.scalar.activation(
            EK[j][:, off:off + w], ps, AF.Exp, bias=cv['bk'][j]))
        if ch == 0:
            # LN1's sb1 apply chain hides under sb0's 8 panel groups
            gemm('wk', ink, k_cb, sbs=(0,), split_first=True)
            gemm('wk', ink, k_cb, sbs=(1,))
        else:
            gemm('wk', ink, k_cb)

        # ---- v/r mixes now (they only need U), so the DVE's scan block
        # owns the whole v+r GEMM window ----
        inv = mk_mix(U, 'mixv', 'mv')
        inr = mk_mix(U, 'mixr', 'mr', RO)

        # ---- v GEMM -> EKV = EK * v ----
        EKV = [ap.tile([128, C], F16, tag=f'ekv{i}', name=f'ekv{i}_{ch}')
               for i in range(NT)]

        def v_cb(j, sb, ps, off, w):
            vt = scr.tile([128, BS], F16, tag='vt', name='vt')
            nc.scalar.activation(vt[:, 0:w], ps, AF.Identity, bias=cv['bv'][j])
            nc.vector.tensor_mul(EKV[j][:, off:off + w],
                                 EK[j][:, off:off + w], vt[:, 0:w])
        gemm('wv', inv, v_cb)

        # ---- WKV scans (fp32 internal state; wkvr reuses the U slots) ----
        wkvr = [ap.tile([128, C], F16, tag=f'u{i}', name=f'wr{i}_{ch}')
                for i in range(NT)]
        for i in range(NT):
            A = wkvp.tile([128, C + 1], F16, tag='A', name='A')
            B = wkvp.tile([128, C + 1], F16, tag='B', name='B')
            if ch != 0:
                # ch0's num/den never read col 0 (so=H-1>0): skip the copies
                nc.vector.tensor_copy(A[:, 0:1], Acol[i][:])
                nc.vector.tensor_copy(B[:, 0:1], Bcol[i][:])
            if ch == 0:
                # warmup segment, then zero core-0's carry at the boundary
                ewb = cv['ew'][i].broadcast_to([128, H])
                nc.vector.tensor_tensor_scan(A[:, 1:H + 1], ewb,
                                             EKV[i][:, 0:H],
                                             Acol[i][:], OP.mult, OP.add)
                nc.vector.tensor_tensor_scan(B[:, 1:H + 1], ewb,
                                             EK[i][:, 0:H],
                                             Bcol[i][:], OP.mult, OP.add)
                nc.vector.tensor_mul(A[:, H:H + 1], A[:, H:H + 1],
                                     cv['cmask'][i])
                nc.vector.tensor_mul(B[:, H:H + 1], B[:, H:H + 1],
                                     cv['cmask'][i])
                ewb2 = cv['ew'][i].broadcast_to([128, C - H])
                nc.vector.tensor_tensor_scan(A[:, H + 1:C + 1], ewb2,
                                             EKV[i][:, H:C],
                                             A[:, H:H + 1], OP.mult, OP.add)
                nc.vector.tensor_tensor_scan(B[:, H + 1:C + 1], ewb2,
                                             EK[i][:, H:C],
                                             B[:, H:H + 1], OP.mult, OP.add)
            else:
                ewb = cv['ew'][i].broadcast_to([128, C])
                nc.vector.tensor_tensor_scan(A[:, 1:C + 1], ewb, EKV[i][:],
                                             A[:, 0:1], OP.mult, OP.add)
                nc.vector.tensor_tensor_scan(B[:, 1:C + 1], ewb, EK[i][:],
                                             B[:, 0:1], OP.mult, OP.add)
            nc.vector.tensor_copy(Acol[i][:], A[:, C:C + 1])
            nc.vector.tensor_copy(Bcol[i][:], B[:, C:C + 1])
            so = H - 1 if ch == 0 else 0   # wkv needed at output cols only
            sw = C - so
            num = wkvp.tile([128, sw], F16, tag='num', name='num')
            nc.vector.scalar_tensor_tensor(num[:], EKV[i][:, so:C],
                                           cv['eu'][i],
                                           A[:, so:C], OP.mult, OP.add)
            den = wkvp.tile([128, sw], F32, tag='den', name='den', bufs=1)
            nc.vector.scalar_tensor_tensor(den[:], EK[i][:, so:C],
                                           cv['eu'][i],
                                           B[:, so:C], OP.mult, OP.add)
            rec = wkvp.tile([128, sw], F32, tag='rec', name='rec', bufs=1)
            nc.vector.reciprocal_approx_fast(rec[:], den[:])
            nc.vector.tensor_mul(wkvr[i][:, so:C], num[:], rec[:])

        # ---- r GEMM (sb-outer: Wr streams twice; mr-sb1 only finishes
        # after the v GEMM's last mv read frees its slot) -> rsig ----
        rsig = [ap.tile([128, C], F16, tag=f'rs{i}', name=f'rs{i}_{ch}')
                for i in range(NT)]
        r_cb = (lambda j, sb, ps, off, w: nc.scalar.activation(
            rsig[j][:, off:off + w], ps, AF.Sigmoid, bias=cv['br'][j]))
        gemm('wr', inr, r_cb, ranges=RO, sbs=(0,))
        gemm('wr', inr, r_cb, ranges=RO, sbs=(1,))

        so0 = H - 1 if ch == 0 else 0
        for i in range(NT):
            nc.vector.tensor_mul(wkvr[i][:, so0:BS], wkvr[i][:, so0:BS],
                                 rsig[i][:, so0:BS])
        for i in range(NT):
            nc.vector.tensor_mul(wkvr[i][:, BS:C], wkvr[i][:, BS:C],
                                 rsig[i][:, BS:C])

        # prefetch the ln/exp act table while the r GEMM runs, so LN2's
        # rstd chain doesn't pay a LoadActFuncSet
        lnpre = rows.tile([128, 1], F32, tag='lnpre', name='lnpre')
        nc.scalar.activation(lnpre[:], ones[:], AF.Ln)

        # ---- atto GEMM (sb-outer: Wo streams twice) -> rz = x + wkv@Wo+bo,
        # with each sub-block's LN2+fmixk emitted under the next one's
        # matmuls ----
        rz = [ap.tile([128, C], F16, tag=f'rz{i}', name=f'rz{i}_{ch}')
              for i in range(NT)]
        if ch == 0:
            # o-GEMM skips warmup cols; zero them so LN2's per-token stats
            # read defined values (those tokens' U2 is never consumed)
            for i in range(NT):
                nc.vector.memset(rz[i][:, 0:H - 1], 0.0)
        rzs = [[rz[i][:, sb * BS:(sb + 1) * BS] for i in range(NT)]
               for sb in range(2)]
        wkvrs = [[wkvr[i][:, off:off + w] for i in range(NT)]
                 for sb, (off, w) in enumerate(RO)]

        def o_cb(j, sb, ps, off, w):
            oa = scr.tile([128, BS], F16, tag='oa', name='oa')
            nc.scalar.activation(oa[:, 0:w], ps, AF.Identity, bias=cv['bo'][j])
            xsl = xall_c[ch][sb][j]
            nc.vector.tensor_add(rz[j][:, off:off + w], oa[:, 0:w],
                                 xsl[:, off - sb * BS:off - sb * BS + w])

        U2 = [ap.tile([128, C + 1], F16, tag=f'u2_{i}', name=f'u2{i}_{ch}')
              for i in range(NT)]
        for i in range(NT):
            nc.vector.tensor_copy(U2[i][:, 0:1], U2col[i][:])
        fmx = [None, None]   # fmx[sb] = [fmixk tiles, fmixr tiles]
        gemm('wo', wkvrs, o_cb, ranges=RO, sbs=(0,))
        fmx[0] = ln_sb(rzs[0], U2, 0, mixes=[('fmixk', 'fk', RF),
                                             ('fmixr', 'fr', RF)])
        gemm('wo', wkvrs, o_cb, ranges=RO, sbs=(1,))
        fmx[1] = ln_sb(rzs[1], U2, 1, mixes=[('fmixk', 'fk', RF),
                                             ('fmixr', 'fr', RF)])
        for i in range(NT):
            nc.vector.tensor_copy(U2col[i][:], U2[i][:, C:C + 1])
        fki = [fmx[0][0], fmx[1][0]]
        fri = [fmx[0][1], fmx[1][1]]

        # ---- FFN, sb-major: each sub-block's fk/fr/fv run back-to-back so
        # the other sub-block's LN2 chain has an 87us window to hide in ----
        kf2 = [ap.tile([128, C], F16, tag=f'ek{i}', name=f'kf{i}_{ch}')
               for i in range(NT)]
        rf = [ap.tile([128, C], F16, tag=f'ekv{i}', name=f'rf{i}_{ch}')
              for i in range(NT)]
        kf2s = [[kf2[i][:, off:off + w] for i in range(NT)]
                for sb, (off, w) in enumerate(RF)]

        def fk_cb(j, sb, ps, off, w):
            kf = scr.tile([128, BS], F16, tag='kf', name='kf')
            nc.scalar.activation(kf[:, 0:w], ps, AF.Relu, bias=cv['bfk'][j])
            nc.scalar.activation(kf2[j][:, off:off + w], kf[:, 0:w], AF.Square)

        fr_cb = (lambda j, sb, ps, off, w: nc.scalar.activation(
            rf[j][:, off:off + w], ps, AF.Sigmoid, bias=cv['bfr'][j]))

        def fv_cb(j, sb, ps, off, w):
            t3 = scr.tile([128, BS], F16, tag='t3', name='t3')
            nc.scalar.activation(t3[:, 0:w], ps, AF.Identity, bias=cv['bfv'][j])
            t4 = scr.tile([128, BS], F16, tag='t4', name='t4')
            nc.vector.tensor_mul(t4[:, 0:w], t3[:, 0:w], rf[j][:, off:off + w])
            ot = scr.tile([128, BS], F16, tag='ot', name='ot')
            nc.vector.tensor_add(ot[:, 0:w], t4[:, 0:w], rz[j][:, off:off + w])
            t0 = ch * C + off
            nc.sync.dma_start(outTt[j, :, t0:t0 + w], ot[:, 0:w])

        gemm('wfk', fki, fk_cb, ranges=RF, sbs=(0,), split_first=True)
        gemm('wfr', fri, fr_cb, ranges=RF, sbs=(0,))
        gemm('wfv', kf2s, fv_cb, ranges=RF, sbs=(0,))
        # next chunk's front half: its DVE chain hides under fk1/fr1/fv1
        if ch + 1 < NCH:
            front = att_front(ch + 1)
        gemm('wfk', fki, fk_cb, ranges=RF, sbs=(1,))
        gemm('wfr', fri, fr_cb, ranges=RF, sbs=(1,))
        gemm('wfv', kf2s, fv_cb, ranges=RF, sbs=(1,))


def prep_inputs(inputs):
    f32, f16 = np.float32, np.float16
    x = np.asarray(inputs['x'], f32)
    g1, b1 = np.asarray(inputs['ln1_g'], f32), np.asarray(inputs['ln1_b'], f32)
    g2, b2 = np.asarray(inputs['ln2_g'], f32), np.asarray(inputs['ln2_b'], f32)
    W, Bv = {}, {}
    for key, nm, g, b in [('wk', 'attk', g1, b1), ('wv', 'attv', g1, b1),
                          ('wr', 'attr', g1, b1), ('wfk', 'ffnk', g2, b2),
                          ('wfr', 'ffnr', g2, b2)]:
        w = np.asarray(inputs[nm + '_w'], f32)
        W[key] = np.ascontiguousarray((w * g[None, :]).T)
        Bv[key] = (np.asarray(inputs[nm + '_b'], f32) + w @ b).astype(f32)
    for key, nm in [('wo', 'atto'), ('wfv', 'ffnv')]:
        w = np.asarray(inputs[nm + '_w'], f32)
        W[key] = np.ascontiguousarray(w.T)
        Bv[key] = np.asarray(inputs[nm + '_b'], f32)
    Wp = {}
    for key, wt in W.items():
        wp = wt.astype(f16).reshape(NT, 128, NJG, JQ * 128)
        Wp[key] = np.ascontiguousarray(
            wp.transpose(2, 1, 0, 3).reshape(NJG, 128, NT * JQ * 128))
    bmap = dict(zip(BNAMES, ['wk', 'wv', 'wr', 'wo', 'wfk', 'wfv', 'wfr']))
    mixes = {'mixk': inputs['attmixk'], 'mixv': inputs['attmixv'],
             'mixr': inputs['attmixr'], 'fmixk': inputs['ffnmixk'],
             'fmixr': inputs['ffnmixr']}
    ew = np.exp(-np.exp(np.asarray(inputs['time_decay'], f32))).astype(f32)
    eu = np.exp(np.asarray(inputs['time_first'], f32)).astype(f32)
    xt = np.ascontiguousarray(x.T)

    def colmat(vec):
        return np.asarray(vec, f32).reshape(NT, 128).T  # [128, NT]

    in_maps = []
    for c in range(NCORES):
        s = c * TLOC
        idx = (np.arange(s - H, s + TLOC)) % T
        xc = xt[:, idx].astype(f16)                      # [D, TBUF]
        xp = xc.reshape(NT, 128, 2 * NCH, BS)
        m = {'xP': np.ascontiguousarray(
            xp.transpose(2, 1, 0, 3).reshape(2 * NCH, 128, NT * BS))}
        for k in WNAMES:
            m[k] = Wp[k]
        cvcols = {}
        for k in BNAMES:
            cvcols[k] = colmat(Bv[bmap[k]])
        for k, v in mixes.items():
            cvcols[k] = colmat(v)
        cvcols['ew'] = colmat(ew)
        cvcols['eu'] = colmat(eu)
        cvcols['cmask'] = np.full((128, NT), 0.0 if c == 0 else 1.0, f32)
        m['cvP'] = np.ascontiguousarray(
            np.concatenate([cvcols[n] for n in CVNAMES], axis=1))
        m['onescol'] = np.ones((128, 1), f16)
        m['onesrow'] = np.ones((1, 128), f32)
        in_maps.append(m)
    return in_maps


_CACHED = {}
TRACE = False
LAST = {}


def kernel(**inputs):
    if 'nc' not in _CACHED:
        _CACHED['nc'] = build_kernel()
    nc = _CACHED['nc']
    in_maps = prep_inputs(inputs)
    kw = {}
    if TRACE:
        kw = dict(trace=True, trace_cores=list(range(NCORES)))
    res = run_bass_kernel_spmd(nc, in_maps, list(range(NCORES)), **kw)
    LAST['res'] = res
    parts = []
    for c in range(NCORES):
        oc = np.asarray(res.results[c]['outT'])         # [D, TBUF] f16
        parts.append(oc[:, H:].T.astype(np.float32))
    return np.ascontiguousarray(np.concatenate(parts, axis=0))


if __name__ == '__main__':
    import reference
    inputs = {k: np.asarray(v) for k, v in reference.setup_inputs().items()}
    out = kernel(**inputs)
    print('out', out.shape, out.dtype)


# revision 7
# speedup vs baseline: 1.1473x; 1.1473x over previous
"""RWKV block (T=8192, D=2048) on 8 Trainium2 NeuronCores.

Data-parallel over the sequence: 1024 tokens/core plus a 64-token
recomputed warmup prefix (power-decay attention forgets at e^{-|w|} per
step, |w|>=0.6, so the truncated WKV state is exact at fp16 precision).
Core 0's warmup is the wrapped tail x[T-64:], and its scan carry is
zeroed at the warmup/main boundary by a cmask column multiply between two
scan segments. Each core's 1088 tokens run in 2 chunks of 544,
phase-major (LN1 -> k/v/r GEMMs -> WKV scans -> atto -> LN2 -> FFN
GEMMs) so each weight streams at most twice per chunk; all GEMM operands
are fp16 (1 PE cycle/row at any size, half the DMA of fp32r) with fp32
PSUM accumulation, and weights/x/constants are host-packed so one DMA
covers a whole panel group.

Latency engineering from the timeline cost model + HW traces: LayerNorm
applies are all-fp16 in SBUF (2x DVE rate) with mean/rstd broadcast via
K=1 ones-matmuls and converted once per sub-block on the Act engine;
stats PSUM is double-buffered; per-tile lerp "mix" tiles are emitted
fused behind each LN apply; the GEMM following each LN runs sub-block-
outer (its weight streams twice) so the second sub-block's apply chain
hides under the first's matmuls, with per-4-tile split guards pacing the
first chain; chunk ch+1's x-DMA + LN1 + mixk are emitted inside ch's FFN
phases; Ln/Exp share one activation table (no LoadActFuncSet ping-pong);
warmup-only columns are skipped by the r/atto/FFN GEMMs.
"""
import sys
if '/opt/trn_rl_repo' not in sys.path:
    sys.path.insert(0, '/opt/trn_rl_repo')

from contextlib import ExitStack
import numpy as np

import concourse.bass as bass
import concourse.tile as tile
from concourse import bacc, mybir
from concourse.bass import _add_dep_helper
from concourse.bass_utils import run_bass_kernel_spmd

F32 = mybir.dt.float32
F32R = mybir.dt.float32r
F16 = mybir.dt.float16
AF = mybir.ActivationFunctionType
OP = mybir.AluOpType

D = 2048
T = 8192
NCORES = 8
TLOC = T // NCORES          # 1024 main tokens per core
H = 64                      # warmup tokens
TBUF = H + TLOC             # 1088
NCH = 2                     # chunks per core
C = TBUF // NCH             # 544 tokens per chunk
BS = C // 2                 # 272-token GEMM sub-block (PSUM bank = 512 f32)
NT = D // 128               # 16 partition tiles
JQ = 2                      # j-tiles per weight panel group (256 out chans)
NJG = NT // JQ              # 8 panel groups per GEMM

WNAMES = ['wk', 'wv', 'wr', 'wo', 'wfk', 'wfv', 'wfr']
BNAMES = ['bk', 'bv', 'br', 'bo', 'bfk', 'bfv', 'bfr']
VNAMES = ['mixk', 'mixv', 'mixr', 'fmixk', 'fmixr', 'ew', 'eu', 'cmask']
CVNAMES = BNAMES + VNAMES


def _prefer_combined_act_table():
    """Steer the act-table chooser so Ln and Exp both resolve to
    natural_log_exp_and_others (one table for the rstd chain AND the EK
    exponentials) instead of ping-ponging LoadActFuncSet (1.28us each).
    Set order/names/indices are preserved — walrus still sees the original
    act_info.json ids — we only hide exp/ln from the smaller sets so the
    first set that satisfies them is the combined one."""
    import functools
    import concourse.hw_specs as hw_specs
    if getattr(bacc, '_act_tables_tuned', False):
        return
    orig = hw_specs.get_activation_tables

    @functools.cache
    def tuned(arch):
        t = {k: set(v) for k, v in orig(arch).items()}
        for name, s in t.items():
            if name == 'natural_log_exp_and_others':
                continue
            s.discard(mybir.ActivationFunctionType.Exp)
            s.discard(mybir.ActivationFunctionType.Ln)
        return t

    bacc.get_activation_tables = tuned
    bacc._act_tables_tuned = True


def build_kernel():
    _prefer_combined_act_table()
    nc = bacc.Bacc()
    xP = nc.declare_dram_parameter('xP', [2 * NCH, 128, NT * BS], F16,
                                   isOutput=False)
    cvP = nc.declare_dram_parameter('cvP', [128, len(CVNAMES) * NT], F32,
                                    isOutput=False)
    onescol = nc.declare_dram_parameter('onescol', [128, 1], F16, isOutput=False)
    onesrow = nc.declare_dram_parameter('onesrow', [1, 128], F32R, isOutput=False)
    wd = {n: nc.declare_dram_parameter(n, [NJG, 128, NT * JQ * 128], F16,
                                       isOutput=False)
          for n in WNAMES}
    outT = nc.declare_dram_parameter('outT', [D, TBUF], F16, isOutput=True)
    outTt = outT.rearrange('(n p) t -> n p t', p=128)

    with tile.TileContext(nc) as tc:
        with ExitStack() as ctx:
            kern(ctx, tc, xP, wd, cvP, outTt, onescol, onesrow)
    nc.compile()
    return nc


def kern(ctx, tc, xP, wd, cvP, outTt, onescol, onesrow):
    nc = tc.nc

    cons = ctx.enter_context(tc.tile_pool(name='cons', bufs=1))
    colp = ctx.enter_context(tc.tile_pool(name='colp', bufs=1))
    wpool = ctx.enter_context(tc.tile_pool(name='wpool', bufs=2))
    ap = ctx.enter_context(tc.tile_pool(name='ap', bufs=1))
    mixp = ctx.enter_context(tc.tile_pool(name='mixp', bufs=3))
    xop = ctx.enter_context(tc.tile_pool(name='xop', bufs=2))
    wkvp = ctx.enter_context(tc.tile_pool(name='wkvp', bufs=2))
    scr = ctx.enter_context(tc.tile_pool(name='scr', bufs=2))
    rows = ctx.enter_context(tc.tile_pool(name='rows', bufs=2))
    psg = ctx.enter_context(tc.tile_pool(name='psg', bufs=4, space='PSUM'))
    pss = ctx.enter_context(tc.tile_pool(name='pss', bufs=2, space='PSUM'))

    # ones first (tiny), then x for chunk 0; constants behind (packed DMA)
    xall_c = {}

    def load_x(ch):
        xts = [xop.tile([128, NT * BS], F16, tag='xall', name=f'xall{ch}{sb}')
               for sb in range(2)]
        for sb in range(2):
            nc.sync.dma_start(xts[sb][:], xP[ch * 2 + sb])
        xall_c[ch] = [[xts[sb][:, i * BS:(i + 1) * BS] for i in range(NT)]
                      for sb in range(2)]

    ones = cons.tile([128, 1], F16, tag='ones', name='ones')
    nc.sync.dma_start(ones[:], onescol[:])
    load_x(0)
    cvt = cons.tile([128, len(CVNAMES) * NT], F32, tag='cvt', name='cvt')
    nc.sync.dma_start(cvt[:], cvP[:])
    cv = {}
    for ni, n in enumerate(CVNAMES):
        cv[n] = [cvt[:, ni * NT + i:ni * NT + i + 1] for i in range(NT)]
    ones_row = cons.tile([1, 128], F32R, tag='ones_row', name='ones_row')
    nc.sync.dma_start(ones_row[:], onesrow[:])
    # trigger the (single) act-table load under the x DMA
    lnp0 = cons.tile([128, 1], F32, tag='lnp0', name='lnp0')
    nc.scalar.activation(lnp0[:], ones[:], AF.Ln)

    def pe_guard(aps):
        """Single-wait carrier for fused-LDW matmul chains (see v2)."""
        eng = nc.tensor
        inst = mybir.InstNoOp(
            name=nc.get_next_instruction_name(),
            text_hint='pe_guard', bass_nofuse=True,
            ins=[eng.lower_ap(a) for a in aps])
        return eng.add_instruction(inst)

    FULL = [(0, BS), (BS, BS)]

    def gemm(wname, rhs_sb, out_cb, ranges=None, split_first=False,
             sbs=(0, 1), wtag='wt'):
        """out[j, sb] = sum_kt w[kt, j].T @ rhs_sb[sb][kt] over ranges[sb].
        Guards are per (j0, sb) so sb0 chains start before sb1 rhs exists;
        with split_first, the first chain's guards are interleaved per
        4-tile group so the PE paces along the producing DVE chain.
        sbs selects which sub-blocks to emit (for sb-outer phases)."""
        if ranges is None:
            ranges = FULL
        for j0 in range(NJG):
            wt = wpool.tile([128, NT * JQ * 128], F16, tag=wtag,
                            name=f'wt_{wname}{j0}')
            nc.sync.dma_start(wt[:], wd[wname][j0])
            for sb in sbs:
                off, w = ranges[sb]
                split = split_first and j0 == 0 and sb == sbs[0]
                guards = [None] * 4
                if not split:
                    guards = [pe_guard([wt[:]] + [rhs_sb[sb][kt]
                                                  for kt in range(NT)])] * 4
                for jj in range(JQ):
                    pt = psg.tile([128, w], F32, tag='ps', name='ps')
                    for kt in range(NT):
                        if split and jj == 0 and kt % 4 == 0:
                            guards[kt // 4] = pe_guard(
                                [wt[:]] + [rhs_sb[sb][k2]
                                           for k2 in range(kt, kt + 4)])
                        lo = kt * JQ * 128 + jj * 128
                        mm = nc.tensor.matmul(
                            pt[:], wt[:, lo:lo + 128], rhs_sb[sb][kt],
                            start=(kt == 0), stop=(kt == NT - 1))
                        _add_dep_helper(mm.ins, guards[kt // 4].ins,
                                        sync=False, reason='order after guard')
                    out_cb(j0 * JQ + jj, sb, pt[:], off, w)

    def ln_stats(xs):
        """Per-token mean/rstd via ones-matmuls; returns fp16 SBUF
        broadcast tiles (s16, m16) [128, BS]."""
        ps_s = pss.tile([1, BS], F32, tag='st0', name='st0')
        ps_q = pss.tile([1, BS], F32, tag='st1', name='st1')
        sq0 = scr.tile([128, BS], F16, tag='sq', name='sq', bufs=4)
        nc.scalar.activation(sq0[:], xs[0], AF.Square)
        guard = pe_guard(list(xs) + [sq0[:], ones[:]])
        for kt in range(NT):
            if kt == 0:
                sq = sq0
            else:
                sq = scr.tile([128, BS], F16, tag='sq', name='sq', bufs=4)
                nc.scalar.activation(sq[:], xs[kt], AF.Square)
            mm = nc.tensor.matmul(ps_s[:], ones[:], xs[kt],
                                  start=(kt == 0), stop=(kt == NT - 1))
            _add_dep_helper(mm.ins, guard.ins, sync=False, reason='g')
            mm2 = nc.tensor.matmul(ps_q[:], ones[:], sq[:],
                                   start=(kt == 0), stop=(kt == NT - 1))
            _add_dep_helper(mm2.ins, guard.ins, sync=False, reason='g')
        mean = rows.tile([1, BS], F32R, tag='mean', name='mean')
        var = rows.tile([1, BS], F32, tag='var', name='var')
        m2 = rows.tile([1, BS], F32, tag='m2', name='m2')
        nc.vector.tensor_scalar_mul(mean[:], ps_s[:], 1.0 / D)
        nc.vector.tensor_scalar_mul(var[:], ps_q[:], 1.0 / D)
        nc.vector.tensor_mul(m2[:], mean[:], mean[:])
        nc.vector.tensor_sub(var[:], var[:], m2[:])
        nc.vector.tensor_scalar_add(var[:], var[:], 1e-5)
        lnv = rows.tile([1, BS], F32, tag='lnv', name='lnv')
        nc.scalar.activation(lnv[:], var[:], AF.Ln)
        rstd = rows.tile([1, BS], F32R, tag='rstd', name='rstd')
        nc.scalar.activation(rstd[:], lnv[:], AF.Exp, scale=-0.5)
        s_b = pss.tile([128, BS], F32, tag='st0', name='s_b')
        m_b = pss.tile([128, BS], F32, tag='st1', name='m_b')
        guard2 = pe_guard([rstd[:], mean[:], ones_row[:]])
        mmb = nc.tensor.matmul(s_b[:], ones_row[:], rstd[:], start=True, stop=True)
        _add_dep_helper(mmb.ins, guard2.ins, sync=False, reason='g2')
        mmb2 = nc.tensor.matmul(m_b[:], ones_row[:], mean[:], start=True, stop=True)
        _add_dep_helper(mmb2.ins, guard2.ins, sync=False, reason='g2')
        # PSUM f32 -> SBUF fp16 once (Act engine), so applies run 2x on DVE
        s16 = scr.tile([128, BS], F16, tag='s16', name='s16')
        m16 = scr.tile([128, BS], F16, tag='m16', name='m16')
        nc.scalar.activation(s16[:], s_b[:], AF.Copy)
        nc.scalar.activation(m16[:], m_b[:], AF.Copy)
        return s16, m16

    def mix_one(Ub, mixname, tagp, i, off, w):
        """d = U[t]-U[t-1], mt = U[t-1] + mix*d over cols [off, off+w)."""
        d = scr.tile([128, w], F16, tag='d1', name=f'd{tagp}', bufs=4)
        nc.vector.tensor_sub(d[:], Ub[i][:, 1 + off:1 + off + w],
                             Ub[i][:, off:off + w])
        mt = mixp.tile([128, w], F16, tag=f'mix{i}', name=f'{tagp}{i}')
        nc.vector.scalar_tensor_tensor(
            mt[:], d[:], cv[mixname][i],
            Ub[i][:, off:off + w], OP.mult, OP.add)
        return mt[:]

    def ln_sb(xs_i, Ub, sb, mix=None):
        """One sub-block of a LayerNorm: stats + all-fp16 applies, with the
        per-tile lerp fused right behind each apply when mix is given."""
        mts = [None] * NT
        s16, m16 = ln_stats(xs_i)
        for i in range(NT):
            t1 = scr.tile([128, BS], F16, tag='ut', name='ut', bufs=4)
            nc.vector.tensor_sub(t1[:], xs_i[i], m16[:])
            nc.vector.tensor_mul(Ub[i][:, 1 + sb * BS:1 + (sb + 1) * BS],
                                 t1[:], s16[:])
            if mix is not None:
                mixname, tagp, ranges = mix
                off, w = ranges[sb]
                mts[i] = mix_one(Ub, mixname, tagp, i, off, w)
        return mts

    def ln_to(xs_sb, Ub, UcolT, mix=None):
        """Full LayerNorm into U buffer [128, C+1] (lead col from UcolT)."""
        for i in range(NT):
            nc.vector.tensor_copy(Ub[i][:, 0:1], UcolT[i][:])
        mts = [ln_sb(xs_sb[sb], Ub, sb, mix) for sb in range(2)]
        for i in range(NT):
            nc.vector.tensor_copy(UcolT[i][:], Ub[i][:, C:C + 1])
        return mts

    def mk_mix(Ub, mixname, tagp, ranges=None):
        """Per-sb lerp tiles for a whole phase (non-latency-critical)."""
        if ranges is None:
            ranges = FULL
        return [[mix_one(Ub, mixname, tagp, i, off, w) for i in range(NT)]
                for sb, (off, w) in enumerate(ranges)]

    # persistent cross-chunk state
    Ucol = [colp.tile([128, 1], F16, tag=f'uc{i}', name=f'uc{i}')
            for i in range(NT)]
    U2col = [colp.tile([128, 1], F16, tag=f'u2c{i}', name=f'u2c{i}')
             for i in range(NT)]
    Acol = [colp.tile([128, 1], F16, tag=f'acl{i}', name=f'acl{i}')
            for i in range(NT)]
    Bcol = [colp.tile([128, 1], F16, tag=f'bcl{i}', name=f'bcl{i}')
            for i in range(NT)]
    for i in range(NT):
        nc.vector.memset(Ucol[i][:], 0.0)
        nc.vector.memset(U2col[i][:], 0.0)
        nc.vector.memset(Acol[i][:], 0.0)
        nc.vector.memset(Bcol[i][:], 0.0)

    def att_front(ch):
        """x-DMA + LN1 + fused mixk for chunk ch (emitted early for ch>0)."""
        if ch not in xall_c:
            load_x(ch)
        U = [ap.tile([128, C + 1], F16, tag=f'u{i}', name=f'u{i}_{ch}')
             for i in range(NT)]
        ink = ln_to(xall_c[ch], U, Ucol, mix=('mixk', 'mk', FULL))
        return U, ink

    front = att_front(0)

    for ch in range(NCH):
        U, ink = front
        xs_sb = xall_c[ch]
        # warmup cols (chunk 0, sb 0 only) are needed by k/v (scan history)
        # and by rz/U2 at the last warmup col; r/atto skip cols < H-1 and
        # the FFN skips cols < H.
        RO = [(H - 1, BS - H + 1), (BS, BS)] if ch == 0 else FULL
        RF = [(H, BS - H), (BS, BS)] if ch == 0 else FULL

        # ---- k GEMM -> EK = exp(k) ----
        EK = [ap.tile([128, C], F16, tag=f'ek{i}', name=f'ek{i}_{ch}')
              for i in range(NT)]
        k_cb = (lambda j, sb, ps, off, w: nc.scalar.activation(
            EK[j][:, off:off + w], ps, AF.Exp, bias=cv['bk'][j]))
        if ch == 0:
            # LN1's sb1 apply chain hides under sb0's 8 panel groups
            gemm('wk', ink, k_cb, sbs=(0,), split_first=True)
            gemm('wk', ink, k_cb, sbs=(1,))
        else:
            gemm('wk', ink, k_cb)

        # ---- v GEMM -> EKV = EK * v ----
        inv = mk_mix(U, 'mixv', 'mv')
        EKV = [ap.tile([128, C], F16, tag=f'ekv{i}', name=f'ekv{i}_{ch}')
               for i in range(NT)]

        def v_cb(j, sb, ps, off, w):
            vt = scr.tile([128, BS], F16, tag='vt', name='vt')
            nc.scalar.activation(vt[:, 0:w], ps, AF.Identity, bias=cv['bv'][j])
            nc.vector.tensor_mul(EKV[j][:, off:off + w],
                                 EK[j][:, off:off + w], vt[:, 0:w])
        gemm('wv', inv, v_cb)

        # ---- r GEMM -> rsig ----
        inr = mk_mix(U, 'mixr', 'mr', RO)
        rsig = [ap.tile([128, C], F16, tag=f'rs{i}', name=f'rs{i}_{ch}')
                for i in range(NT)]
        gemm('wr', inr,
             lambda j, sb, ps, off, w: nc.scalar.activation(
                 rsig[j][:, off:off + w], ps, AF.Sigmoid, bias=cv['br'][j]),
             ranges=RO)

        # ---- WKV scans (fp32 internal state; wkvr reuses the U slots) ----
        wkvr = [ap.tile([128, C], F16, tag=f'u{i}', name=f'wr{i}_{ch}')
                for i in range(NT)]
        for i in range(NT):
            A = wkvp.tile([128, C + 1], F16, tag='A', name='A')
            B = wkvp.tile([128, C + 1], F16, tag='B', name='B')
            nc.vector.tensor_copy(A[:, 0:1], Acol[i][:])
            nc.vector.tensor_copy(B[:, 0:1], Bcol[i][:])
            if ch == 0:
                # warmup segment, then zero core-0's carry at the boundary
                ewb = cv['ew'][i].broadcast_to([128, H])
                nc.vector.tensor_tensor_scan(A[:, 1:H + 1], ewb,
                                             EKV[i][:, 0:H],
                                             A[:, 0:1], OP.mult, OP.add)
                nc.vector.tensor_tensor_scan(B[:, 1:H + 1], ewb,
                                             EK[i][:, 0:H],
                                             B[:, 0:1], OP.mult, OP.add)
                nc.vector.tensor_mul(A[:, H:H + 1], A[:, H:H + 1],
                                     cv['cmask'][i])
                nc.vector.tensor_mul(B[:, H:H + 1], B[:, H:H + 1],
                                     cv['cmask'][i])
                ewb2 = cv['ew'][i].broadcast_to([128, C - H])
                nc.vector.tensor_tensor_scan(A[:, H + 1:C + 1], ewb2,
                                             EKV[i][:, H:C],
                                             A[:, H:H + 1], OP.mult, OP.add)
                nc.vector.tensor_tensor_scan(B[:, H + 1:C + 1], ewb2,
                                             EK[i][:, H:C],
                                             B[:, H:H + 1], OP.mult, OP.add)
            else:
                ewb = cv['ew'][i].broadcast_to([128, C])
                nc.vector.tensor_tensor_scan(A[:, 1:C + 1], ewb, EKV[i][:],
                                             A[:, 0:1], OP.mult, OP.add)
                nc.vector.tensor_tensor_scan(B[:, 1:C + 1], ewb, EK[i][:],
                                             B[:, 0:1], OP.mult, OP.add)
            nc.vector.tensor_copy(Acol[i][:], A[:, C:C + 1])
            nc.vector.tensor_copy(Bcol[i][:], B[:, C:C + 1])
            so = H - 1 if ch == 0 else 0   # wkv needed at output cols only
            sw = C - so
            num = wkvp.tile([128, sw], F16, tag='num', name='num')
            nc.vector.scalar_tensor_tensor(num[:], EKV[i][:, so:C],
                                           cv['eu'][i],
                                           A[:, so:C], OP.mult, OP.add)
            den = wkvp.tile([128, sw], F32, tag='den', name='den', bufs=1)
            nc.vector.scalar_tensor_tensor(den[:], EK[i][:, so:C],
                                           cv['eu'][i],
                                           B[:, so:C], OP.mult, OP.add)
            rec = wkvp.tile([128, sw], F32, tag='rec', name='rec', bufs=1)
            nc.vector.reciprocal_approx_fast(rec[:], den[:])
            nc.vector.tensor_mul(wkvr[i][:, so:C], num[:], rec[:])
        so0 = H - 1 if ch == 0 else 0
        for i in range(NT):
            nc.vector.tensor_mul(wkvr[i][:, so0:C], wkvr[i][:, so0:C],
                                 rsig[i][:, so0:C])

        # prefetch the ln/exp act table while the r GEMM runs, so LN2's
        # rstd chain doesn't pay a LoadActFuncSet
        lnpre = rows.tile([128, 1], F32, tag='lnpre', name='lnpre')
        nc.scalar.activation(lnpre[:], ones[:], AF.Ln)

        # ---- atto GEMM (sb-outer: Wo streams twice) -> rz = x + wkv@Wo+bo,
        # with each sub-block's LN2+fmixk emitted under the next one's
        # matmuls ----
        rz = [ap.tile([128, C], F16, tag=f'rz{i}', name=f'rz{i}_{ch}')
              for i in range(NT)]
        if ch == 0:
            # o-GEMM skips warmup cols; zero them so LN2's per-token stats
            # read defined values (those tokens' U2 is never consumed)
            for i in range(NT):
                nc.vector.memset(rz[i][:, 0:H - 1], 0.0)
        rzs = [[rz[i][:, sb * BS:(sb + 1) * BS] for i in range(NT)]
               for sb in range(2)]
        wkvrs = [[wkvr[i][:, off:off + w] for i in range(NT)]
                 for sb, (off, w) in enumerate(RO)]

        def o_cb(j, sb, ps, off, w):
            oa = scr.tile([128, BS], F16, tag='oa', name='oa')
            nc.scalar.activation(oa[:, 0:w], ps, AF.Identity, bias=cv['bo'][j])
            xsl = xall_c[ch][sb][j]
            nc.vector.tensor_add(rz[j][:, off:off + w], oa[:, 0:w],
                                 xsl[:, off - sb * BS:off - sb * BS + w])

        U2 = [ap.tile([128, C + 1], F16, tag=f'u2_{i}', name=f'u2{i}_{ch}')
              for i in range(NT)]
        for i in range(NT):
            nc.vector.tensor_copy(U2[i][:, 0:1], U2col[i][:])
        fki = [None, None]
        gemm('wo', wkvrs, o_cb, ranges=RO, sbs=(0,))
        fki[0] = ln_sb(rzs[0], U2, 0, mix=('fmixk', 'fk', RF))
        gemm('wo', wkvrs, o_cb, ranges=RO, sbs=(1,))
        fki[1] = ln_sb(rzs[1], U2, 1, mix=('fmixk', 'fk', RF))
        for i in range(NT):
            nc.vector.tensor_copy(U2col[i][:], U2[i][:, C:C + 1])

        # ---- ffnk GEMM -> kf2 = relu(kf)^2 (reuses ek slots) ----
        kf2 = [ap.tile([128, C], F16, tag=f'ek{i}', name=f'kf{i}_{ch}')
               for i in range(NT)]

        def fk_cb(j, sb, ps, off, w):
            kf = scr.tile([128, BS], F16, tag='kf', name='kf')
            nc.scalar.activation(kf[:, 0:w], ps, AF.Relu, bias=cv['bfk'][j])
            nc.vector.tensor_mul(kf2[j][:, off:off + w], kf[:, 0:w],
                                 kf[:, 0:w])
        gemm('wfk', fki, fk_cb, ranges=RF, sbs=(0,), split_first=True)
        gemm('wfk', fki, fk_cb, ranges=RF, sbs=(1,))

        # ---- ffnr GEMM -> rf (reuses ekv slots) ----
        fri = mk_mix(U2, 'fmixr', 'fr', RF)

        # next chunk's front half, emitted under the FFN GEMMs. Must come
        # after fri (U2's last reader) so U(ch+1)'s writes — same tile slot —
        # never wait on DVE work queued behind them.
        if ch + 1 < NCH:
            front = att_front(ch + 1)
        rf = [ap.tile([128, C], F16, tag=f'ekv{i}', name=f'rf{i}_{ch}')
              for i in range(NT)]
        gemm('wfr', fri,
             lambda j, sb, ps, off, w: nc.scalar.activation(
                 rf[j][:, off:off + w], ps, AF.Sigmoid, bias=cv['bfr'][j]),
             ranges=RF)

        # ---- ffnv GEMM -> out = (kf2 @ Wfv + bfv) * rf + rz ----
        kf2s = [[kf2[i][:, off:off + w] for i in range(NT)]
                for sb, (off, w) in enumerate(RF)]

        def fv_cb(j, sb, ps, off, w):
            t3 = scr.tile([128, BS], F16, tag='t3', name='t3')
            nc.scalar.activation(t3[:, 0:w], ps, AF.Identity, bias=cv['bfv'][j])
            t4 = scr.tile([128, BS], F16, tag='t4', name='t4')
            nc.vector.tensor_mul(t4[:, 0:w], t3[:, 0:w], rf[j][:, off:off + w])
            ot = scr.tile([128, BS], F16, tag='ot', name='ot')
            nc.vector.tensor_add(ot[:, 0:w], t4[:, 0:w], rz[j][:, off:off + w])
            t0 = ch * C + off
            nc.sync.dma_start(outTt[j, :, t0:t0 + w], ot[:, 0:w])
        gemm('wfv', kf2s, fv_cb, ranges=RF)


def prep_inputs(inputs):
    f32, f16 = np.float32, np.float16
    x = np.asarray(inputs['x'], f32)
    g1, b1 = np.asarray(inputs['ln1_g'], f32), np.asarray(inputs['ln1_b'], f32)
    g2, b2 = np.asarray(inputs['ln2_g'], f32), np.asarray(inputs['ln2_b'], f32)
    W, Bv = {}, {}
    for key, nm, g, b in [('wk', 'attk', g1, b1), ('wv', 'attv', g1, b1),
                          ('wr', 'attr', g1, b1), ('wfk', 'ffnk', g2, b2),
                          ('wfr', 'ffnr', g2, b2)]:
        w = np.asarray(inputs[nm + '_w'], f32)
        W[key] = np.ascontiguousarray((w * g[None, :]).T)
        Bv[key] = (np.asarray(inputs[nm + '_b'], f32) + w @ b).astype(f32)
    for key, nm in [('wo', 'atto'), ('wfv', 'ffnv')]:
        w = np.asarray(inputs[nm + '_w'], f32)
        W[key] = np.ascontiguousarray(w.T)
        Bv[key] = np.asarray(inputs[nm + '_b'], f32)
    Wp = {}
    for key, wt in W.items():
        wp = wt.astype(f16).reshape(NT, 128, NJG, JQ * 128)
        Wp[key] = np.ascontiguousarray(
            wp.transpose(2, 1, 0, 3).reshape(NJG, 128, NT * JQ * 128))
    bmap = dict(zip(BNAMES, ['wk', 'wv', 'wr', 'wo', 'wfk', 'wfv', 'wfr']))
    mixes = {'mixk': inputs['attmixk'], 'mixv': inputs['attmixv'],
             'mixr': inputs['attmixr'], 'fmixk': inputs['ffnmixk'],
             'fmixr': inputs['ffnmixr']}
    ew = np.exp(-np.exp(np.asarray(inputs['time_decay'], f32))).astype(f32)
    eu = np.exp(np.asarray(inputs['time_first'], f32)).astype(f32)
    xt = np.ascontiguousarray(x.T)

    def colmat(vec):
        return np.asarray(vec, f32).reshape(NT, 128).T  # [128, NT]

    in_maps = []
    for c in range(NCORES):
        s = c * TLOC
        idx = (np.arange(s - H, s + TLOC)) % T
        xc = xt[:, idx].astype(f16)                      # [D, TBUF]
        xp = xc.reshape(NT, 128, 2 * NCH, BS)
        m = {'xP': np.ascontiguousarray(
            xp.transpose(2, 1, 0, 3).reshape(2 * NCH, 128, NT * BS))}
        for k in WNAMES:
            m[k] = Wp[k]
        cvcols = {}
        for k in BNAMES:
            cvcols[k] = colmat(Bv[bmap[k]])
        for k, v in mixes.items():
            cvcols[k] = colmat(v)
        cvcols['ew'] = colmat(ew)
        cvcols['eu'] = colmat(eu)
        cvcols['cmask'] = np.full((128, NT), 0.0 if c == 0 else 1.0, f32)
        m['cvP'] = np.ascontiguousarray(
            np.concatenate([cvcols[n] for n in CVNAMES], axis=1))
        m['onescol'] = np.ones((128, 1), f16)
        m['onesrow'] = np.ones((1, 128), f32)
        in_maps.append(m)
    return in_maps


_CACHED = {}
TRACE = False
LAST = {}


def kernel(**inputs):
    if 'nc' not in _CACHED:
        _CACHED['nc'] = build_kernel()
    nc = _CACHED['nc']
    in_maps = prep_inputs(inputs)
    kw = {}
    if TRACE:
        kw = dict(trace=True, trace_cores=list(range(NCORES)))
    res = run_bass_kernel_spmd(nc, in_maps, list(range(NCORES)), **kw)
    LAST['res'] = res
    parts = []
    for c in range(NCORES):
        oc = np.asarray(res.results[c]['outT'])         # [D, TBUF] f16
        parts.append(oc[:, H:].T.astype(np.float32))
    return np.ascontiguousarray(np.concatenate(parts, axis=0))


if __name__ == '__main__':
    import reference
    inputs = {k: np.asarray(v) for k, v in reference.setup_inputs().items()}
    out = kernel(**inputs)
    print('out', out.shape, out.dtype)


# revision 8
# speedup vs baseline: 1.2627x; 1.1005x over previous
"""RWKV block (T=8192, D=2048) on 8 Trainium2 NeuronCores.

Data-parallel over the sequence: 1024 tokens/core plus a 64-token
recomputed warmup prefix (power-decay attention forgets at e^{-|w|} per
step, |w|>=0.6, so the truncated WKV state is exact at fp16 precision).
Core 0's warmup is the wrapped tail x[T-64:], and its scan carry is
zeroed at the warmup/main boundary by a cmask column multiply between two
scan segments. Each core's 1088 tokens run in 2 chunks of 544,
phase-major (LN1 -> k/v/r GEMMs -> WKV scans -> atto -> LN2 -> FFN
GEMMs) so each weight streams at most twice per chunk; all GEMM operands
are fp16 (1 PE cycle/row at any size, half the DMA of fp32r) with fp32
PSUM accumulation, and weights/x/constants are host-packed so one DMA
covers a whole panel group.

Latency engineering from the timeline cost model + HW traces: LayerNorm
applies are all-fp16 in SBUF (2x DVE rate) with mean/rstd broadcast via
K=1 ones-matmuls and converted once per sub-block on the Act engine;
stats PSUM is double-buffered; per-tile lerp "mix" tiles are emitted
fused behind each LN apply; the GEMM following each LN runs sub-block-
outer (its weight streams twice) so the second sub-block's apply chain
hides under the first's matmuls, with per-4-tile split guards pacing the
first chain; chunk ch+1's x-DMA + LN1 + mixk are emitted inside ch's FFN
phases; Ln/Exp share one activation table (no LoadActFuncSet ping-pong);
warmup-only columns are skipped by the r/atto/FFN GEMMs and the WKV
epilogue.
"""
import sys
if '/opt/trn_rl_repo' not in sys.path:
    sys.path.insert(0, '/opt/trn_rl_repo')

from contextlib import ExitStack
import numpy as np

import concourse.bass as bass
import concourse.tile as tile
from concourse import bacc, mybir
from concourse.bass import _add_dep_helper
from concourse.bass_utils import run_bass_kernel_spmd

F32 = mybir.dt.float32
F32R = mybir.dt.float32r
F16 = mybir.dt.float16
AF = mybir.ActivationFunctionType
OP = mybir.AluOpType

D = 2048
T = 8192
NCORES = 8
TLOC = T // NCORES          # 1024 main tokens per core
H = 64                      # warmup tokens
TBUF = H + TLOC             # 1088
NCH = 2                     # chunks per core
C = TBUF // NCH             # 544 tokens per chunk
BS = C // 2                 # 272-token GEMM sub-block (PSUM bank = 512 f32)
NT = D // 128               # 16 partition tiles
JQ = 2                      # j-tiles per weight panel group (256 out chans)
NJG = NT // JQ              # 8 panel groups per GEMM

WNAMES = ['wk', 'wv', 'wr', 'wo', 'wfk', 'wfv', 'wfr']
BNAMES = ['bk', 'bv', 'br', 'bo', 'bfk', 'bfv', 'bfr']
VNAMES = ['mixk', 'mixv', 'mixr', 'fmixk', 'fmixr', 'ew', 'eu', 'cmask']
CVNAMES = BNAMES + VNAMES


def _prefer_combined_act_table():
    """Steer the act-table chooser so Ln and Exp both resolve to
    natural_log_exp_and_others (one table for the rstd chain AND the EK
    exponentials) instead of ping-ponging LoadActFuncSet (1.28us each).
    Set order/names/indices are preserved — walrus still sees the original
    act_info.json ids — we only hide exp/ln from the smaller sets so the
    first set that satisfies them is the combined one."""
    import functools
    import concourse.hw_specs as hw_specs
    if getattr(bacc, '_act_tables_tuned', False):
        return
    orig = hw_specs.get_activation_tables

    @functools.cache
    def tuned(arch):
        t = {k: set(v) for k, v in orig(arch).items()}
        for name, s in t.items():
            if name == 'natural_log_exp_and_others':
                continue
            s.discard(mybir.ActivationFunctionType.Exp)
            s.discard(mybir.ActivationFunctionType.Ln)
        return t

    bacc.get_activation_tables = tuned
    bacc._act_tables_tuned = True


def build_kernel():
    _prefer_combined_act_table()
    nc = bacc.Bacc()
    xP = nc.declare_dram_parameter('xP', [2 * NCH, 128, NT * BS], F16,
                                   isOutput=False)
    cvP = nc.declare_dram_parameter('cvP', [128, len(CVNAMES) * NT], F32,
                                    isOutput=False)
    onescol = nc.declare_dram_parameter('onescol', [128, 1], F16, isOutput=False)
    onesrow = nc.declare_dram_parameter('onesrow', [1, 128], F32R, isOutput=False)
    wd = {n: nc.declare_dram_parameter(n, [NJG, 128, NT * JQ * 128], F16,
                                       isOutput=False)
          for n in WNAMES}
    outT = nc.declare_dram_parameter('outT', [D, TBUF], F16, isOutput=True)
    outTt = outT.rearrange('(n p) t -> n p t', p=128)

    with tile.TileContext(nc) as tc:
        with ExitStack() as ctx:
            kern(ctx, tc, xP, wd, cvP, outTt, onescol, onesrow)
    nc.compile()
    return nc


def kern(ctx, tc, xP, wd, cvP, outTt, onescol, onesrow):
    nc = tc.nc

    cons = ctx.enter_context(tc.tile_pool(name='cons', bufs=1))
    colp = ctx.enter_context(tc.tile_pool(name='colp', bufs=1))
    wpool = ctx.enter_context(tc.tile_pool(name='wpool', bufs=2))
    ap = ctx.enter_context(tc.tile_pool(name='ap', bufs=1))
    mixp = ctx.enter_context(tc.tile_pool(name='mixp', bufs=4))
    xop = ctx.enter_context(tc.tile_pool(name='xop', bufs=2))
    wkvp = ctx.enter_context(tc.tile_pool(name='wkvp', bufs=2))
    scr = ctx.enter_context(tc.tile_pool(name='scr', bufs=2))
    rows = ctx.enter_context(tc.tile_pool(name='rows', bufs=2))
    psg = ctx.enter_context(tc.tile_pool(name='psg', bufs=4, space='PSUM'))
    pss = ctx.enter_context(tc.tile_pool(name='pss', bufs=2, space='PSUM'))

    # ones first (tiny), then x for chunk 0; constants behind (packed DMA)
    xall_c = {}

    def load_x(ch):
        xts = [xop.tile([128, NT * BS], F16, tag='xall', name=f'xall{ch}{sb}')
               for sb in range(2)]
        for sb in range(2):
            nc.sync.dma_start(xts[sb][:], xP[ch * 2 + sb])
        xall_c[ch] = [[xts[sb][:, i * BS:(i + 1) * BS] for i in range(NT)]
                      for sb in range(2)]

    ones = cons.tile([128, 1], F16, tag='ones', name='ones')
    nc.sync.dma_start(ones[:], onescol[:])
    load_x(0)
    cvt = cons.tile([128, len(CVNAMES) * NT], F32, tag='cvt', name='cvt')
    nc.sync.dma_start(cvt[:], cvP[:])
    cv = {}
    for ni, n in enumerate(CVNAMES):
        cv[n] = [cvt[:, ni * NT + i:ni * NT + i + 1] for i in range(NT)]
    ones_row = cons.tile([1, 128], F32R, tag='ones_row', name='ones_row')
    nc.sync.dma_start(ones_row[:], onesrow[:])
    # trigger the (single) act-table load under the x DMA
    lnp0 = cons.tile([128, 1], F32, tag='lnp0', name='lnp0')
    nc.scalar.activation(lnp0[:], ones[:], AF.Ln)

    def pe_guard(aps):
        """Single-wait carrier for fused-LDW matmul chains (see v2)."""
        eng = nc.tensor
        inst = mybir.InstNoOp(
            name=nc.get_next_instruction_name(),
            text_hint='pe_guard', bass_nofuse=True,
            ins=[eng.lower_ap(a) for a in aps])
        return eng.add_instruction(inst)

    FULL = [(0, BS), (BS, BS)]

    def gemm(wname, rhs_sb, out_cb, ranges=None, split_first=False,
             sbs=(0, 1), wtag='wt'):
        """out[j, sb] = sum_kt w[kt, j].T @ rhs_sb[sb][kt] over ranges[sb].
        Guards are per (j0, sb) so sb0 chains start before sb1 rhs exists;
        with split_first, the first chain's guards are interleaved per
        4-tile group so the PE paces along the producing DVE chain.
        sbs selects which sub-blocks to emit (for sb-outer phases)."""
        if ranges is None:
            ranges = FULL
        for j0 in range(NJG):
            wt = wpool.tile([128, NT * JQ * 128], F16, tag=wtag,
                            name=f'wt_{wname}{j0}')
            nc.sync.dma_start(wt[:], wd[wname][j0])
            for sb in sbs:
                off, w = ranges[sb]
                split = split_first and j0 == 0 and sb == sbs[0]
                guards = [None] * 4
                if not split:
                    guards = [pe_guard([wt[:]] + [rhs_sb[sb][kt]
                                                  for kt in range(NT)])] * 4
                for jj in range(JQ):
                    pt = psg.tile([128, w], F32, tag='ps', name='ps')
                    for kt in range(NT):
                        if split and jj == 0 and kt % 4 == 0:
                            guards[kt // 4] = pe_guard(
                                [wt[:]] + [rhs_sb[sb][k2]
                                           for k2 in range(kt, kt + 4)])
                        lo = kt * JQ * 128 + jj * 128
                        mm = nc.tensor.matmul(
                            pt[:], wt[:, lo:lo + 128], rhs_sb[sb][kt],
                            start=(kt == 0), stop=(kt == NT - 1))
                        _add_dep_helper(mm.ins, guards[kt // 4].ins,
                                        sync=False, reason='order after guard')
                    out_cb(j0 * JQ + jj, sb, pt[:], off, w)

    def ln_stats(xs):
        """Per-token mean/rstd via ones-matmuls; returns fp16 SBUF
        broadcast tiles (s16, m16) [128, BS]."""
        ps_s = pss.tile([1, BS], F32, tag='st0', name='st0')
        ps_q = pss.tile([1, BS], F32, tag='st1', name='st1')
        sq0 = scr.tile([128, BS], F16, tag='sq', name='sq', bufs=4)
        nc.scalar.activation(sq0[:], xs[0], AF.Square)
        guard = pe_guard(list(xs) + [sq0[:], ones[:]])
        for kt in range(NT):
            if kt == 0:
                sq = sq0
            else:
                sq = scr.tile([128, BS], F16, tag='sq', name='sq', bufs=4)
                nc.scalar.activation(sq[:], xs[kt], AF.Square)
            mm = nc.tensor.matmul(ps_s[:], ones[:], xs[kt],
                                  start=(kt == 0), stop=(kt == NT - 1))
            _add_dep_helper(mm.ins, guard.ins, sync=False, reason='g')
            mm2 = nc.tensor.matmul(ps_q[:], ones[:], sq[:],
                                   start=(kt == 0), stop=(kt == NT - 1))
            _add_dep_helper(mm2.ins, guard.ins, sync=False, reason='g')
        mean = rows.tile([1, BS], F32R, tag='mean', name='mean')
        var = rows.tile([1, BS], F32, tag='var', name='var')
        m2 = rows.tile([1, BS], F32, tag='m2', name='m2')
        nc.vector.tensor_scalar_mul(mean[:], ps_s[:], 1.0 / D)
        nc.vector.tensor_scalar_mul(var[:], ps_q[:], 1.0 / D)
        nc.vector.tensor_mul(m2[:], mean[:], mean[:])
        nc.vector.tensor_sub(var[:], var[:], m2[:])
        nc.vector.tensor_scalar_add(var[:], var[:], 1e-5)
        lnv = rows.tile([1, BS], F32, tag='lnv', name='lnv')
        nc.scalar.activation(lnv[:], var[:], AF.Ln)
        rstd = rows.tile([1, BS], F32R, tag='rstd', name='rstd')
        nc.scalar.activation(rstd[:], lnv[:], AF.Exp, scale=-0.5)
        s_b = pss.tile([128, BS], F32, tag='st0', name='s_b')
        m_b = pss.tile([128, BS], F32, tag='st1', name='m_b')
        guard2 = pe_guard([rstd[:], mean[:], ones_row[:]])
        mmb = nc.tensor.matmul(s_b[:], ones_row[:], rstd[:], start=True, stop=True)
        _add_dep_helper(mmb.ins, guard2.ins, sync=False, reason='g2')
        mmb2 = nc.tensor.matmul(m_b[:], ones_row[:], mean[:], start=True, stop=True)
        _add_dep_helper(mmb2.ins, guard2.ins, sync=False, reason='g2')
        # PSUM f32 -> SBUF fp16 once (Act engine), so applies run 2x on DVE
        s16 = scr.tile([128, BS], F16, tag='s16', name='s16')
        m16 = scr.tile([128, BS], F16, tag='m16', name='m16')
        nc.scalar.activation(s16[:], s_b[:], AF.Copy)
        nc.scalar.activation(m16[:], m_b[:], AF.Copy)
        return s16, m16

    def mix_one(Ub, mixname, tagp, i, off, w):
        """d = U[t]-U[t-1], mt = U[t-1] + mix*d over cols [off, off+w)."""
        d = scr.tile([128, w], F16, tag='d1', name=f'd{tagp}', bufs=4)
        nc.vector.tensor_sub(d[:], Ub[i][:, 1 + off:1 + off + w],
                             Ub[i][:, off:off + w])
        mt = mixp.tile([128, w], F16, tag=f'mix{i}', name=f'{tagp}{i}')
        nc.vector.scalar_tensor_tensor(
            mt[:], d[:], cv[mixname][i],
            Ub[i][:, off:off + w], OP.mult, OP.add)
        return mt[:]

    def ln_sb(xs_i, Ub, sb, mix=None):
        """One sub-block of a LayerNorm: stats + all-fp16 applies, with the
        per-tile lerp fused right behind each apply when mix is given."""
        mts = [None] * NT
        s16, m16 = ln_stats(xs_i)
        for i in range(NT):
            t1 = scr.tile([128, BS], F16, tag='ut', name='ut', bufs=4)
            nc.vector.tensor_sub(t1[:], xs_i[i], m16[:])
            nc.vector.tensor_mul(Ub[i][:, 1 + sb * BS:1 + (sb + 1) * BS],
                                 t1[:], s16[:])
            if mix is not None:
                mixname, tagp, ranges = mix
                off, w = ranges[sb]
                mts[i] = mix_one(Ub, mixname, tagp, i, off, w)
        return mts

    def ln_to(xs_sb, Ub, UcolT, mix=None):
        """Full LayerNorm into U buffer [128, C+1] (lead col from UcolT)."""
        for i in range(NT):
            nc.vector.tensor_copy(Ub[i][:, 0:1], UcolT[i][:])
        mts = [ln_sb(xs_sb[sb], Ub, sb, mix) for sb in range(2)]
        for i in range(NT):
            nc.vector.tensor_copy(UcolT[i][:], Ub[i][:, C:C + 1])
        return mts

    def mk_mix(Ub, mixname, tagp, ranges=None):
        """Per-sb lerp tiles for a whole phase (non-latency-critical)."""
        if ranges is None:
            ranges = FULL
        return [[mix_one(Ub, mixname, tagp, i, off, w) for i in range(NT)]
                for sb, (off, w) in enumerate(ranges)]

    # persistent cross-chunk state
    Ucol = [colp.tile([128, 1], F16, tag=f'uc{i}', name=f'uc{i}')
            for i in range(NT)]
    U2col = [colp.tile([128, 1], F16, tag=f'u2c{i}', name=f'u2c{i}')
             for i in range(NT)]
    Acol = [colp.tile([128, 1], F16, tag=f'acl{i}', name=f'acl{i}')
            for i in range(NT)]
    Bcol = [colp.tile([128, 1], F16, tag=f'bcl{i}', name=f'bcl{i}')
            for i in range(NT)]
    for i in range(NT):
        nc.vector.memset(Ucol[i][:], 0.0)
        nc.vector.memset(U2col[i][:], 0.0)
        nc.vector.memset(Acol[i][:], 0.0)
        nc.vector.memset(Bcol[i][:], 0.0)

    def att_front(ch):
        """x-DMA + LN1 + fused mixk for chunk ch (emitted early for ch>0)."""
        if ch not in xall_c:
            load_x(ch)
        U = [ap.tile([128, C + 1], F16, tag=f'u{i}', name=f'u{i}_{ch}')
             for i in range(NT)]
        ink = ln_to(xall_c[ch], U, Ucol, mix=('mixk', 'mk', FULL))
        return U, ink

    front = att_front(0)

    for ch in range(NCH):
        U, ink = front
        xs_sb = xall_c[ch]
        # warmup cols (chunk 0, sb 0 only) are needed by k/v (scan history)
        # and by rz/U2 at the last warmup col; r/atto skip cols < H-1 and
        # the FFN skips cols < H.
        RO = [(H - 1, BS - H + 1), (BS, BS)] if ch == 0 else FULL
        RF = [(H, BS - H), (BS, BS)] if ch == 0 else FULL

        # ---- k GEMM -> EK = exp(k) ----
        EK = [ap.tile([128, C], F16, tag=f'ek{i}', name=f'ek{i}_{ch}')
              for i in range(NT)]
        k_cb = (lambda j, sb, ps, off, w: nc.scalar.activation(
            EK[j][:, off:off + w], ps, AF.Exp, bias=cv['bk'][j]))
        if ch == 0:
            # LN1's sb1 apply chain hides under sb0's 8 panel groups
            gemm('wk', ink, k_cb, sbs=(0,), split_first=True)
            gemm('wk', ink, k_cb, sbs=(1,))
        else:
            gemm('wk', ink, k_cb)

        # ---- v GEMM -> EKV = EK * v ----
        inv = mk_mix(U, 'mixv', 'mv')
        EKV = [ap.tile([128, C], F16, tag=f'ekv{i}', name=f'ekv{i}_{ch}')
               for i in range(NT)]

        def v_cb(j, sb, ps, off, w):
            vt = scr.tile([128, BS], F16, tag='vt', name='vt')
            nc.scalar.activation(vt[:, 0:w], ps, AF.Identity, bias=cv['bv'][j])
            nc.vector.tensor_mul(EKV[j][:, off:off + w],
                                 EK[j][:, off:off + w], vt[:, 0:w])
        gemm('wv', inv, v_cb)

        # ---- r GEMM -> rsig ----
        inr = mk_mix(U, 'mixr', 'mr', RO)
        rsig = [ap.tile([128, C], F16, tag=f'rs{i}', name=f'rs{i}_{ch}')
                for i in range(NT)]
        gemm('wr', inr,
             lambda j, sb, ps, off, w: nc.scalar.activation(
                 rsig[j][:, off:off + w], ps, AF.Sigmoid, bias=cv['br'][j]),
             ranges=RO)

        # ---- WKV scans (fp32 internal state; wkvr reuses the U slots) ----
        wkvr = [ap.tile([128, C], F16, tag=f'u{i}', name=f'wr{i}_{ch}')
                for i in range(NT)]
        for i in range(NT):
            A = wkvp.tile([128, C + 1], F16, tag='A', name='A')
            B = wkvp.tile([128, C + 1], F16, tag='B', name='B')
            if ch != 0:
                # ch0's num/den never read col 0 (so=H-1>0): skip the copies
                nc.vector.tensor_copy(A[:, 0:1], Acol[i][:])
                nc.vector.tensor_copy(B[:, 0:1], Bcol[i][:])
            if ch == 0:
                # warmup segment, then zero core-0's carry at the boundary
                ewb = cv['ew'][i].broadcast_to([128, H])
                nc.vector.tensor_tensor_scan(A[:, 1:H + 1], ewb,
                                             EKV[i][:, 0:H],
                                             Acol[i][:], OP.mult, OP.add)
                nc.vector.tensor_tensor_scan(B[:, 1:H + 1], ewb,
                                             EK[i][:, 0:H],
                                             Bcol[i][:], OP.mult, OP.add)
                nc.vector.tensor_mul(A[:, H:H + 1], A[:, H:H + 1],
                                     cv['cmask'][i])
                nc.vector.tensor_mul(B[:, H:H + 1], B[:, H:H + 1],
                                     cv['cmask'][i])
                ewb2 = cv['ew'][i].broadcast_to([128, C - H])
                nc.vector.tensor_tensor_scan(A[:, H + 1:C + 1], ewb2,
                                             EKV[i][:, H:C],
                                             A[:, H:H + 1], OP.mult, OP.add)
                nc.vector.tensor_tensor_scan(B[:, H + 1:C + 1], ewb2,
                                             EK[i][:, H:C],
                                             B[:, H:H + 1], OP.mult, OP.add)
            else:
                ewb = cv['ew'][i].broadcast_to([128, C])
                nc.vector.tensor_tensor_scan(A[:, 1:C + 1], ewb, EKV[i][:],
                                             A[:, 0:1], OP.mult, OP.add)
                nc.vector.tensor_tensor_scan(B[:, 1:C + 1], ewb, EK[i][:],
                                             B[:, 0:1], OP.mult, OP.add)
            nc.vector.tensor_copy(Acol[i][:], A[:, C:C + 1])
            nc.vector.tensor_copy(Bcol[i][:], B[:, C:C + 1])
            so = H - 1 if ch == 0 else 0   # wkv needed at output cols only
            sw = C - so
            num = wkvp.tile([128, sw], F16, tag='num', name='num')
            nc.vector.scalar_tensor_tensor(num[:], EKV[i][:, so:C],
                                           cv['eu'][i],
                                           A[:, so:C], OP.mult, OP.add)
            den = wkvp.tile([128, sw], F32, tag='den', name='den', bufs=1)
            nc.vector.scalar_tensor_tensor(den[:], EK[i][:, so:C],
                                           cv['eu'][i],
                                           B[:, so:C], OP.mult, OP.add)
            rec = wkvp.tile([128, sw], F32, tag='rec', name='rec', bufs=1)
            nc.vector.reciprocal_approx_fast(rec[:], den[:])
            nc.vector.tensor_mul(wkvr[i][:, so:C], num[:], rec[:])
        so0 = H - 1 if ch == 0 else 0
        for i in range(NT):
            nc.vector.tensor_mul(wkvr[i][:, so0:BS], wkvr[i][:, so0:BS],
                                 rsig[i][:, so0:BS])
        for i in range(NT):
            nc.vector.tensor_mul(wkvr[i][:, BS:C], wkvr[i][:, BS:C],
                                 rsig[i][:, BS:C])

        # prefetch the ln/exp act table while the r GEMM runs, so LN2's
        # rstd chain doesn't pay a LoadActFuncSet
        lnpre = rows.tile([128, 1], F32, tag='lnpre', name='lnpre')
        nc.scalar.activation(lnpre[:], ones[:], AF.Ln)

        # ---- atto GEMM (sb-outer: Wo streams twice) -> rz = x + wkv@Wo+bo,
        # with each sub-block's LN2+fmixk emitted under the next one's
        # matmuls ----
        rz = [ap.tile([128, C], F16, tag=f'rz{i}', name=f'rz{i}_{ch}')
              for i in range(NT)]
        if ch == 0:
            # o-GEMM skips warmup cols; zero them so LN2's per-token stats
            # read defined values (those tokens' U2 is never consumed)
            for i in range(NT):
                nc.vector.memset(rz[i][:, 0:H - 1], 0.0)
        rzs = [[rz[i][:, sb * BS:(sb + 1) * BS] for i in range(NT)]
               for sb in range(2)]
        wkvrs = [[wkvr[i][:, off:off + w] for i in range(NT)]
                 for sb, (off, w) in enumerate(RO)]

        def o_cb(j, sb, ps, off, w):
            oa = scr.tile([128, BS], F16, tag='oa', name='oa')
            nc.scalar.activation(oa[:, 0:w], ps, AF.Identity, bias=cv['bo'][j])
            xsl = xall_c[ch][sb][j]
            nc.vector.tensor_add(rz[j][:, off:off + w], oa[:, 0:w],
                                 xsl[:, off - sb * BS:off - sb * BS + w])

        U2 = [ap.tile([128, C + 1], F16, tag=f'u2_{i}', name=f'u2{i}_{ch}')
              for i in range(NT)]
        for i in range(NT):
            nc.vector.tensor_copy(U2[i][:, 0:1], U2col[i][:])
        fki = [None, None]
        gemm('wo', wkvrs, o_cb, ranges=RO, sbs=(0,))
        fki[0] = ln_sb(rzs[0], U2, 0, mix=('fmixk', 'fk', RF))
        gemm('wo', wkvrs, o_cb, ranges=RO, sbs=(1,))
        fki[1] = ln_sb(rzs[1], U2, 1, mix=('fmixk', 'fk', RF))
        for i in range(NT):
            nc.vector.tensor_copy(U2col[i][:], U2[i][:, C:C + 1])

        # ---- ffnk GEMM -> kf2 = relu(kf)^2 (reuses ek slots) ----
        kf2 = [ap.tile([128, C], F16, tag=f'ek{i}', name=f'kf{i}_{ch}')
               for i in range(NT)]

        def fk_cb(j, sb, ps, off, w):
            kf = scr.tile([128, BS], F16, tag='kf', name='kf')
            nc.scalar.activation(kf[:, 0:w], ps, AF.Relu, bias=cv['bfk'][j])
            nc.scalar.activation(kf2[j][:, off:off + w], kf[:, 0:w], AF.Square)
        gemm('wfk', fki, fk_cb, ranges=RF, sbs=(0,), split_first=True)
        gemm('wfk', fki, fk_cb, ranges=RF, sbs=(1,))

        # ---- ffnr GEMM -> rf (reuses ekv slots) ----
        fri = mk_mix(U2, 'fmixr', 'fr', RF)

        # next chunk's front half, emitted under the FFN GEMMs. Must come
        # after fri (U2's last reader) so U(ch+1)'s writes — same tile slot —
        # never wait on DVE work queued behind them.
        if ch + 1 < NCH:
            front = att_front(ch + 1)
        rf = [ap.tile([128, C], F16, tag=f'ekv{i}', name=f'rf{i}_{ch}')
              for i in range(NT)]
        gemm('wfr', fri,
             lambda j, sb, ps, off, w: nc.scalar.activation(
                 rf[j][:, off:off + w], ps, AF.Sigmoid, bias=cv['bfr'][j]),
             ranges=RF)

        # ---- ffnv GEMM -> out = (kf2 @ Wfv + bfv) * rf + rz ----
        kf2s = [[kf2[i][:, off:off + w] for i in range(NT)]
                for sb, (off, w) in enumerate(RF)]

        def fv_cb(j, sb, ps, off, w):
            t3 = scr.tile([128, BS], F16, tag='t3', name='t3')
            nc.scalar.activation(t3[:, 0:w], ps, AF.Identity, bias=cv['bfv'][j])
            t4 = scr.tile([128, BS], F16, tag='t4', name='t4')
            nc.vector.tensor_mul(t4[:, 0:w], t3[:, 0:w], rf[j][:, off:off + w])
            ot = scr.tile([128, BS], F16, tag='ot', name='ot')
            nc.vector.tensor_add(ot[:, 0:w], t4[:, 0:w], rz[j][:, off:off + w])
            t0 = ch * C + off
            nc.sync.dma_start(outTt[j, :, t0:t0 + w], ot[:, 0:w])
        gemm('wfv', kf2s, fv_cb, ranges=RF)


def prep_inputs(inputs):
    f32, f16 = np.float32, np.float16
    x = np.asarray(inputs['x'], f32)
    g1, b1 = np.asarray(inputs['ln1_g'], f32), np.asarray(inputs['ln1_b'], f32)
    g2, b2 = np.asarray(inputs['ln2_g'], f32), np.asarray(inputs['ln2_b'], f32)
    W, Bv = {}, {}
    for key, nm, g, b in [('wk', 'attk', g1, b1), ('wv', 'attv', g1, b1),
                          ('wr', 'attr', g1, b1), ('wfk', 'ffnk', g2, b2),
                          ('wfr', 'ffnr', g2, b2)]:
        w = np.asarray(inputs[nm + '_w'], f32)
        W[key] = np.ascontiguousarray((w * g[None, :]).T)
        Bv[key] = (np.asarray(inputs[nm + '_b'], f32) + w @ b).astype(f32)
    for key, nm in [('wo', 'atto'), ('wfv', 'ffnv')]:
        w = np.asarray(inputs[nm + '_w'], f32)
        W[key] = np.ascontiguousarray(w.T)
        Bv[key] = np.asarray(inputs[nm + '_b'], f32)
    Wp = {}
    for key, wt in W.items():
        wp = wt.astype(f16).reshape(NT, 128, NJG, JQ * 128)
        Wp[key] = np.ascontiguousarray(
            wp.transpose(2, 1, 0, 3).reshape(NJG, 128, NT * JQ * 128))
    bmap = dict(zip(BNAMES, ['wk', 'wv', 'wr', 'wo', 'wfk', 'wfv', 'wfr']))
    mixes = {'mixk': inputs['attmixk'], 'mixv': inputs['attmixv'],
             'mixr': inputs['attmixr'], 'fmixk': inputs['ffnmixk'],
             'fmixr': inputs['ffnmixr']}
    ew = np.exp(-np.exp(np.asarray(inputs['time_decay'], f32))).astype(f32)
    eu = np.exp(np.asarray(inputs['time_first'], f32)).astype(f32)
    xt = np.ascontiguousarray(x.T)

    def colmat(vec):
        return np.asarray(vec, f32).reshape(NT, 128).T  # [128, NT]

    in_maps = []
    for c in range(NCORES):
        s = c * TLOC
        idx = (np.arange(s - H, s + TLOC)) % T
        xc = xt[:, idx].astype(f16)                      # [D, TBUF]
        xp = xc.reshape(NT, 128, 2 * NCH, BS)
        m = {'xP': np.ascontiguousarray(
            xp.transpose(2, 1, 0, 3).reshape(2 * NCH, 128, NT * BS))}
        for k in WNAMES:
            m[k] = Wp[k]
        cvcols = {}
        for k in BNAMES:
            cvcols[k] = colmat(Bv[bmap[k]])
        for k, v in mixes.items():
            cvcols[k] = colmat(v)
        cvcols['ew'] = colmat(ew)
        cvcols['eu'] = colmat(eu)
        cvcols['cmask'] = np.full((128, NT), 0.0 if c == 0 else 1.0, f32)
        m['cvP'] = np.ascontiguousarray(
            np.concatenate([cvcols[n] for n in CVNAMES], axis=1))
        m['onescol'] = np.ones((128, 1), f16)
        m['onesrow'] = np.ones((1, 128), f32)
        in_maps.append(m)
    return in_maps


_CACHED = {}
TRACE = False
LAST = {}


def kernel(**inputs):
    if 'nc' not in _CACHED:
        _CACHED['nc'] = build_kernel()
    nc = _CACHED['nc']
    in_maps = prep_inputs(inputs)
    kw = {}
    if TRACE:
        kw = dict(trace=True, trace_cores=list(range(NCORES)))
    res = run_bass_kernel_spmd(nc, in_maps, list(range(NCORES)), **kw)
    LAST['res'] = res
    parts = []
    for c in range(NCORES):
        oc = np.asarray(res.results[c]['outT'])         # [D, TBUF] f16
        parts.append(oc[:, H:].T.astype(np.float32))
    return np.ascontiguousarray(np.concatenate(parts, axis=0))


if __name__ == '__main__':
    import reference
    inputs = {k: np.asarray(v) for k, v in reference.setup_inputs().items()}
    out = kernel(**inputs)
    print('out', out.shape, out.dtype)


# revision 9
# speedup vs baseline: 1.2691x; 1.0051x over previous
"""RWKV block (T=8192, D=2048) on 8 Trainium2 NeuronCores.

Data-parallel over the sequence: 1024 tokens/core plus a 64-token
recomputed warmup prefix (power-decay attention forgets at e^{-|w|} per
step, |w|>=0.6, so the truncated WKV state is exact at fp16 precision).
Core 0's warmup is the wrapped tail x[T-64:], and its scan carry is
zeroed at the warmup/main boundary by a cmask column multiply between two
scan segments. Each core's 1088 tokens run in 2 chunks of 544,
phase-major (LN1 -> k/v/r GEMMs -> WKV scans -> atto -> LN2 -> FFN
GEMMs) so each weight streams at most twice per chunk; all GEMM operands
are fp16 (1 PE cycle/row at any size, half the DMA of fp32r) with fp32
PSUM accumulation, and weights/x/constants are host-packed so one DMA
covers a whole panel group; weight panel groups are triple-buffered so
the next group's DMA hides fully behind the current group's matmuls.

Latency engineering from the timeline cost model + HW traces: LayerNorm
applies are all-fp16 in SBUF (2x DVE rate) with mean/rstd broadcast via
K=1 ones-matmuls and converted once per sub-block on the Act engine;
stats PSUM is double-buffered; per-tile lerp "mix" tiles are emitted
fused behind each LN apply; the GEMM following each LN runs sub-block-
outer (its weight streams twice) so the second sub-block's apply chain
hides under the first's matmuls, with per-4-tile split guards pacing the
first chain; chunk ch+1's x-DMA + LN1 + mixk are emitted inside ch's FFN
phases; Ln/Exp share one activation table (no LoadActFuncSet ping-pong);
warmup-only columns are skipped by the r/atto/FFN GEMMs and the WKV
epilogue.
"""
import sys
if '/opt/trn_rl_repo' not in sys.path:
    sys.path.insert(0, '/opt/trn_rl_repo')

from contextlib import ExitStack
import numpy as np

import concourse.bass as bass
import concourse.tile as tile
from concourse import bacc, mybir
from concourse.bass import _add_dep_helper
from concourse.bass_utils import run_bass_kernel_spmd

F32 = mybir.dt.float32
F32R = mybir.dt.float32r
F16 = mybir.dt.float16
AF = mybir.ActivationFunctionType
OP = mybir.AluOpType

D = 2048
T = 8192
NCORES = 8
TLOC = T // NCORES          # 1024 main tokens per core
H = 64                      # warmup tokens
TBUF = H + TLOC             # 1088
NCH = 2                     # chunks per core
C = TBUF // NCH             # 544 tokens per chunk
BS = C // 2                 # 272-token GEMM sub-block (PSUM bank = 512 f32)
NT = D // 128               # 16 partition tiles
JQ = 2                      # j-tiles per weight panel group (256 out chans)
NJG = NT // JQ              # 8 panel groups per GEMM

WNAMES = ['wk', 'wv', 'wr', 'wo', 'wfk', 'wfv', 'wfr']
BNAMES = ['bk', 'bv', 'br', 'bo', 'bfk', 'bfv', 'bfr']
VNAMES = ['mixk', 'mixv', 'mixr', 'fmixk', 'fmixr', 'ew', 'eu', 'cmask']
CVNAMES = BNAMES + VNAMES


def _prefer_combined_act_table():
    """Steer the act-table chooser so Ln and Exp both resolve to
    natural_log_exp_and_others (one table for the rstd chain AND the EK
    exponentials) instead of ping-ponging LoadActFuncSet (1.28us each).
    Set order/names/indices are preserved — walrus still sees the original
    act_info.json ids — we only hide exp/ln from the smaller sets so the
    first set that satisfies them is the combined one."""
    import functools
    import concourse.hw_specs as hw_specs
    if getattr(bacc, '_act_tables_tuned', False):
        return
    orig = hw_specs.get_activation_tables

    @functools.cache
    def tuned(arch):
        t = {k: set(v) for k, v in orig(arch).items()}
        for name, s in t.items():
            if name == 'natural_log_exp_and_others':
                continue
            s.discard(mybir.ActivationFunctionType.Exp)
            s.discard(mybir.ActivationFunctionType.Ln)
        return t

    bacc.get_activation_tables = tuned
    bacc._act_tables_tuned = True


def build_kernel():
    _prefer_combined_act_table()
    nc = bacc.Bacc()
    xP = nc.declare_dram_parameter('xP', [2 * NCH, 128, NT * BS], F16,
                                   isOutput=False)
    cvP = nc.declare_dram_parameter('cvP', [128, len(CVNAMES) * NT], F32,
                                    isOutput=False)
    onescol = nc.declare_dram_parameter('onescol', [128, 1], F16, isOutput=False)
    onesrow = nc.declare_dram_parameter('onesrow', [1, 128], F32R, isOutput=False)
    wd = {n: nc.declare_dram_parameter(n, [NJG, 128, NT * JQ * 128], F16,
                                       isOutput=False)
          for n in WNAMES}
    outT = nc.declare_dram_parameter('outT', [D, TBUF], F16, isOutput=True)
    outTt = outT.rearrange('(n p) t -> n p t', p=128)

    with tile.TileContext(nc) as tc:
        with ExitStack() as ctx:
            kern(ctx, tc, xP, wd, cvP, outTt, onescol, onesrow)
    nc.compile()
    return nc


def kern(ctx, tc, xP, wd, cvP, outTt, onescol, onesrow):
    nc = tc.nc

    cons = ctx.enter_context(tc.tile_pool(name='cons', bufs=1))
    colp = ctx.enter_context(tc.tile_pool(name='colp', bufs=1))
    wpool = ctx.enter_context(tc.tile_pool(name='wpool', bufs=2))
    ap = ctx.enter_context(tc.tile_pool(name='ap', bufs=1))
    mixp = ctx.enter_context(tc.tile_pool(name='mixp', bufs=4))
    xop = ctx.enter_context(tc.tile_pool(name='xop', bufs=2))
    wkvp = ctx.enter_context(tc.tile_pool(name='wkvp', bufs=2))
    scr = ctx.enter_context(tc.tile_pool(name='scr', bufs=2))
    rows = ctx.enter_context(tc.tile_pool(name='rows', bufs=2))
    psg = ctx.enter_context(tc.tile_pool(name='psg', bufs=4, space='PSUM'))
    pss = ctx.enter_context(tc.tile_pool(name='pss', bufs=2, space='PSUM'))

    # ones first (tiny), then x for chunk 0; constants behind (packed DMA)
    xall_c = {}

    def load_x(ch):
        xts = [xop.tile([128, NT * BS], F16, tag='xall', name=f'xall{ch}{sb}')
               for sb in range(2)]
        for sb in range(2):
            nc.sync.dma_start(xts[sb][:], xP[ch * 2 + sb])
        xall_c[ch] = [[xts[sb][:, i * BS:(i + 1) * BS] for i in range(NT)]
                      for sb in range(2)]

    ones = cons.tile([128, 1], F16, tag='ones', name='ones')
    nc.sync.dma_start(ones[:], onescol[:])
    load_x(0)
    cvt = cons.tile([128, len(CVNAMES) * NT], F32, tag='cvt', name='cvt')
    nc.sync.dma_start(cvt[:], cvP[:])
    cv = {}
    for ni, n in enumerate(CVNAMES):
        cv[n] = [cvt[:, ni * NT + i:ni * NT + i + 1] for i in range(NT)]
    ones_row = cons.tile([1, 128], F32R, tag='ones_row', name='ones_row')
    nc.sync.dma_start(ones_row[:], onesrow[:])
    # trigger the (single) act-table load under the x DMA
    lnp0 = cons.tile([128, 1], F32, tag='lnp0', name='lnp0')
    nc.scalar.activation(lnp0[:], ones[:], AF.Ln)

    def pe_guard(aps):
        """Single-wait carrier for fused-LDW matmul chains (see v2)."""
        eng = nc.tensor
        inst = mybir.InstNoOp(
            name=nc.get_next_instruction_name(),
            text_hint='pe_guard', bass_nofuse=True,
            ins=[eng.lower_ap(a) for a in aps])
        return eng.add_instruction(inst)

    FULL = [(0, BS), (BS, BS)]

    def gemm(wname, rhs_sb, out_cb, ranges=None, split_first=False,
             sbs=(0, 1), wtag='wt'):
        """out[j, sb] = sum_kt w[kt, j].T @ rhs_sb[sb][kt] over ranges[sb].
        Guards are per (j0, sb) so sb0 chains start before sb1 rhs exists;
        with split_first, the first chain's guards are interleaved per
        4-tile group so the PE paces along the producing DVE chain.
        sbs selects which sub-blocks to emit (for sb-outer phases)."""
        if ranges is None:
            ranges = FULL
        for j0 in range(NJG):
            wt = wpool.tile([128, NT * JQ * 128], F16, tag=wtag,
                            name=f'wt_{wname}{j0}')
            nc.sync.dma_start(wt[:], wd[wname][j0])
            for sb in sbs:
                off, w = ranges[sb]
                split = split_first and j0 == 0 and sb == sbs[0]
                guards = [None] * 4
                if not split:
                    guards = [pe_guard([wt[:]] + [rhs_sb[sb][kt]
                                                  for kt in range(NT)])] * 4
                for jj in range(JQ):
                    pt = psg.tile([128, w], F32, tag='ps', name='ps')
                    for kt in range(NT):
                        if split and jj == 0 and kt % 4 == 0:
                            guards[kt // 4] = pe_guard(
                                [wt[:]] + [rhs_sb[sb][k2]
                                           for k2 in range(kt, kt + 4)])
                        lo = kt * JQ * 128 + jj * 128
                        mm = nc.tensor.matmul(
                            pt[:], wt[:, lo:lo + 128], rhs_sb[sb][kt],
                            start=(kt == 0), stop=(kt == NT - 1))
                        _add_dep_helper(mm.ins, guards[kt // 4].ins,
                                        sync=False, reason='order after guard')
                    out_cb(j0 * JQ + jj, sb, pt[:], off, w)

    def ln_stats(xs):
        """Per-token mean/rstd via ones-matmuls; returns fp16 SBUF
        broadcast tiles (s16, m16) [128, BS]."""
        ps_s = pss.tile([1, BS], F32, tag='st0', name='st0')
        ps_q = pss.tile([1, BS], F32, tag='st1', name='st1')
        sq0 = scr.tile([128, BS], F16, tag='sq', name='sq', bufs=4)
        nc.scalar.activation(sq0[:], xs[0], AF.Square)
        guard = pe_guard(list(xs) + [sq0[:], ones[:]])
        for kt in range(NT):
            if kt == 0:
                sq = sq0
            else:
                sq = scr.tile([128, BS], F16, tag='sq', name='sq', bufs=4)
                nc.scalar.activation(sq[:], xs[kt], AF.Square)
            mm = nc.tensor.matmul(ps_s[:], ones[:], xs[kt],
                                  start=(kt == 0), stop=(kt == NT - 1))
            _add_dep_helper(mm.ins, guard.ins, sync=False, reason='g')
            mm2 = nc.tensor.matmul(ps_q[:], ones[:], sq[:],
                                   start=(kt == 0), stop=(kt == NT - 1))
            _add_dep_helper(mm2.ins, guard.ins, sync=False, reason='g')
        mean = rows.tile([1, BS], F32R, tag='mean', name='mean')
        var = rows.tile([1, BS], F32, tag='var', name='var')
        m2 = rows.tile([1, BS], F32, tag='m2', name='m2')
        nc.vector.tensor_scalar_mul(mean[:], ps_s[:], 1.0 / D)
        nc.vector.tensor_scalar_mul(var[:], ps_q[:], 1.0 / D)
        nc.vector.tensor_mul(m2[:], mean[:], mean[:])
        nc.vector.tensor_sub(var[:], var[:], m2[:])
        nc.vector.tensor_scalar_add(var[:], var[:], 1e-5)
        lnv = rows.tile([1, BS], F32, tag='lnv', name='lnv')
        nc.scalar.activation(lnv[:], var[:], AF.Ln)
        rstd = rows.tile([1, BS], F32R, tag='rstd', name='rstd')
        nc.scalar.activation(rstd[:], lnv[:], AF.Exp, scale=-0.5)
        s_b = pss.tile([128, BS], F32, tag='st0', name='s_b')
        m_b = pss.tile([128, BS], F32, tag='st1', name='m_b')
        guard2 = pe_guard([rstd[:], mean[:], ones_row[:]])
        mmb = nc.tensor.matmul(s_b[:], ones_row[:], rstd[:], start=True, stop=True)
        _add_dep_helper(mmb.ins, guard2.ins, sync=False, reason='g2')
        mmb2 = nc.tensor.matmul(m_b[:], ones_row[:], mean[:], start=True, stop=True)
        _add_dep_helper(mmb2.ins, guard2.ins, sync=False, reason='g2')
        # PSUM f32 -> SBUF fp16 once (Act engine), so applies run 2x on DVE
        s16 = scr.tile([128, BS], F16, tag='s16', name='s16')
        m16 = scr.tile([128, BS], F16, tag='m16', name='m16')
        nc.scalar.activation(s16[:], s_b[:], AF.Copy)
        nc.scalar.activation(m16[:], m_b[:], AF.Copy)
        return s16, m16

    def mix_one(Ub, mixname, tagp, i, off, w):
        """d = U[t]-U[t-1], mt = U[t-1] + mix*d over cols [off, off+w)."""
        d = scr.tile([128, w], F16, tag='d1', name=f'd{tagp}', bufs=4)
        nc.vector.tensor_sub(d[:], Ub[i][:, 1 + off:1 + off + w],
                             Ub[i][:, off:off + w])
        mt = mixp.tile([128, w], F16, tag=f'mix{i}', name=f'{tagp}{i}')
        nc.vector.scalar_tensor_tensor(
            mt[:], d[:], cv[mixname][i],
            Ub[i][:, off:off + w], OP.mult, OP.add)
        return mt[:]

    def ln_sb(xs_i, Ub, sb, mix=None):
        """One sub-block of a LayerNorm: stats + all-fp16 applies, with the
        per-tile lerp fused right behind each apply when mix is given."""
        mts = [None] * NT
        s16, m16 = ln_stats(xs_i)
        for i in range(NT):
            t1 = scr.tile([128, BS], F16, tag='ut', name='ut', bufs=4)
            nc.vector.tensor_sub(t1[:], xs_i[i], m16[:])
            nc.vector.tensor_mul(Ub[i][:, 1 + sb * BS:1 + (sb + 1) * BS],
                                 t1[:], s16[:])
            if mix is not None:
                mixname, tagp, ranges = mix
                off, w = ranges[sb]
                mts[i] = mix_one(Ub, mixname, tagp, i, off, w)
        return mts

    def ln_to(xs_sb, Ub, UcolT, mix=None):
        """Full LayerNorm into U buffer [128, C+1] (lead col from UcolT)."""
        for i in range(NT):
            nc.vector.tensor_copy(Ub[i][:, 0:1], UcolT[i][:])
        mts = [ln_sb(xs_sb[sb], Ub, sb, mix) for sb in range(2)]
        for i in range(NT):
            nc.vector.tensor_copy(UcolT[i][:], Ub[i][:, C:C + 1])
        return mts

    def mk_mix(Ub, mixname, tagp, ranges=None):
        """Per-sb lerp tiles for a whole phase (non-latency-critical)."""
        if ranges is None:
            ranges = FULL
        return [[mix_one(Ub, mixname, tagp, i, off, w) for i in range(NT)]
                for sb, (off, w) in enumerate(ranges)]

    # persistent cross-chunk state
    Ucol = [colp.tile([128, 1], F16, tag=f'uc{i}', name=f'uc{i}')
            for i in range(NT)]
    U2col = [colp.tile([128, 1], F16, tag=f'u2c{i}', name=f'u2c{i}')
             for i in range(NT)]
    Acol = [colp.tile([128, 1], F16, tag=f'acl{i}', name=f'acl{i}')
            for i in range(NT)]
    Bcol = [colp.tile([128, 1], F16, tag=f'bcl{i}', name=f'bcl{i}')
            for i in range(NT)]
    for i in range(NT):
        nc.vector.memset(Ucol[i][:], 0.0)
        nc.vector.memset(U2col[i][:], 0.0)
        nc.vector.memset(Acol[i][:], 0.0)
        nc.vector.memset(Bcol[i][:], 0.0)

    def att_front(ch):
        """x-DMA + LN1 + fused mixk for chunk ch (emitted early for ch>0)."""
        if ch not in xall_c:
            load_x(ch)
        U = [ap.tile([128, C + 1], F16, tag=f'u{i}', name=f'u{i}_{ch}')
             for i in range(NT)]
        ink = ln_to(xall_c[ch], U, Ucol, mix=('mixk', 'mk', FULL))
        return U, ink

    front = att_front(0)

    for ch in range(NCH):
        U, ink = front
        xs_sb = xall_c[ch]
        # warmup cols (chunk 0, sb 0 only) are needed by k/v (scan history)
        # and by rz/U2 at the last warmup col; r/atto skip cols < H-1 and
        # the FFN skips cols < H.
        RO = [(H - 1, BS - H + 1), (BS, BS)] if ch == 0 else FULL
        RF = [(H, BS - H), (BS, BS)] if ch == 0 else FULL

        # ---- k GEMM -> EK = exp(k) ----
        EK = [ap.tile([128, C], F16, tag=f'ek{i}', name=f'ek{i}_{ch}')
              for i in range(NT)]
        k_cb = (lambda j, sb, ps, off, w: nc.scalar.activation(
            EK[j][:, off:off + w], ps, AF.Exp, bias=cv['bk'][j]))
        if ch == 0:
            # LN1's sb1 apply chain hides under sb0's 8 panel groups
            gemm('wk', ink, k_cb, sbs=(0,), split_first=True)
            gemm('wk', ink, k_cb, sbs=(1,))
        else:
            gemm('wk', ink, k_cb)

        # ---- v GEMM -> EKV = EK * v ----
        inv = mk_mix(U, 'mixv', 'mv')
        EKV = [ap.tile([128, C], F16, tag=f'ekv{i}', name=f'ekv{i}_{ch}')
               for i in range(NT)]

        def v_cb(j, sb, ps, off, w):
            vt = scr.tile([128, BS], F16, tag='vt', name='vt')
            nc.scalar.activation(vt[:, 0:w], ps, AF.Identity, bias=cv['bv'][j])
            nc.vector.tensor_mul(EKV[j][:, off:off + w],
                                 EK[j][:, off:off + w], vt[:, 0:w])
        gemm('wv', inv, v_cb)

        # ---- r GEMM -> rsig ----
        inr = mk_mix(U, 'mixr', 'mr', RO)
        rsig = [ap.tile([128, C], F16, tag=f'rs{i}', name=f'rs{i}_{ch}')
                for i in range(NT)]
        gemm('wr', inr,
             lambda j, sb, ps, off, w: nc.scalar.activation(
                 rsig[j][:, off:off + w], ps, AF.Sigmoid, bias=cv['br'][j]),
             ranges=RO)

        # ---- WKV scans (fp32 internal state; wkvr reuses the U slots) ----
        wkvr = [ap.tile([128, C], F16, tag=f'u{i}', name=f'wr{i}_{ch}')
                for i in range(NT)]
        for i in range(NT):
            A = wkvp.tile([128, C + 1], F16, tag='A', name='A')
            B = wkvp.tile([128, C + 1], F16, tag='B', name='B')
            if ch != 0:
                # ch0's num/den never read col 0 (so=H-1>0): skip the copies
                nc.vector.tensor_copy(A[:, 0:1], Acol[i][:])
                nc.vector.tensor_copy(B[:, 0:1], Bcol[i][:])
            if ch == 0:
                # warmup segment, then zero core-0's carry at the boundary
                ewb = cv['ew'][i].broadcast_to([128, H])
                nc.vector.tensor_tensor_scan(A[:, 1:H + 1], ewb,
                                             EKV[i][:, 0:H],
                                             Acol[i][:], OP.mult, OP.add)
                nc.vector.tensor_tensor_scan(B[:, 1:H + 1], ewb,
                                             EK[i][:, 0:H],
                                             Bcol[i][:], OP.mult, OP.add)
                nc.vector.tensor_mul(A[:, H:H + 1], A[:, H:H + 1],
                                     cv['cmask'][i])
                nc.vector.tensor_mul(B[:, H:H + 1], B[:, H:H + 1],
                                     cv['cmask'][i])
                ewb2 = cv['ew'][i].broadcast_to([128, C - H])
                nc.vector.tensor_tensor_scan(A[:, H + 1:C + 1], ewb2,
                                             EKV[i][:, H:C],
                                             A[:, H:H + 1], OP.mult, OP.add)
                nc.vector.tensor_tensor_scan(B[:, H + 1:C + 1], ewb2,
                                             EK[i][:, H:C],
                                             B[:, H:H + 1], OP.mult, OP.add)
            else:
                ewb = cv['ew'][i].broadcast_to([128, C])
                nc.vector.tensor_tensor_scan(A[:, 1:C + 1], ewb, EKV[i][:],
                                             A[:, 0:1], OP.mult, OP.add)
                nc.vector.tensor_tensor_scan(B[:, 1:C + 1], ewb, EK[i][:],
                                             B[:, 0:1], OP.mult, OP.add)
            nc.vector.tensor_copy(Acol[i][:], A[:, C:C + 1])
            nc.vector.tensor_copy(Bcol[i][:], B[:, C:C + 1])
            so = H - 1 if ch == 0 else 0   # wkv needed at output cols only
            sw = C - so
            num = wkvp.tile([128, sw], F16, tag='num', name='num')
            nc.vector.scalar_tensor_tensor(num[:], EKV[i][:, so:C],
                                           cv['eu'][i],
                                           A[:, so:C], OP.mult, OP.add)
            den = wkvp.tile([128, sw], F32, tag='den', name='den', bufs=1)
            nc.vector.scalar_tensor_tensor(den[:], EK[i][:, so:C],
                                           cv['eu'][i],
                                           B[:, so:C], OP.mult, OP.add)
            rec = wkvp.tile([128, sw], F32, tag='rec', name='rec', bufs=1)
            nc.vector.reciprocal_approx_fast(rec[:], den[:])
            nc.vector.tensor_mul(wkvr[i][:, so:C], num[:], rec[:])
        so0 = H - 1 if ch == 0 else 0
        for i in range(NT):
            nc.vector.tensor_mul(wkvr[i][:, so0:BS], wkvr[i][:, so0:BS],
                                 rsig[i][:, so0:BS])
        for i in range(NT):
            nc.vector.tensor_mul(wkvr[i][:, BS:C], wkvr[i][:, BS:C],
                                 rsig[i][:, BS:C])

        # prefetch the ln/exp act table while the r GEMM runs, so LN2's
        # rstd chain doesn't pay a LoadActFuncSet
        lnpre = rows.tile([128, 1], F32, tag='lnpre', name='lnpre')
        nc.scalar.activation(lnpre[:], ones[:], AF.Ln)

        # ---- atto GEMM (sb-outer: Wo streams twice) -> rz = x + wkv@Wo+bo,
        # with each sub-block's LN2+fmixk emitted under the next one's
        # matmuls ----
        rz = [ap.tile([128, C], F16, tag=f'rz{i}', name=f'rz{i}_{ch}')
              for i in range(NT)]
        if ch == 0:
            # o-GEMM skips warmup cols; zero them so LN2's per-token stats
            # read defined values (those tokens' U2 is never consumed)
            for i in range(NT):
                nc.vector.memset(rz[i][:, 0:H - 1], 0.0)
        rzs = [[rz[i][:, sb * BS:(sb + 1) * BS] for i in range(NT)]
               for sb in range(2)]
        wkvrs = [[wkvr[i][:, off:off + w] for i in range(NT)]
                 for sb, (off, w) in enumerate(RO)]

        def o_cb(j, sb, ps, off, w):
            oa = scr.tile([128, BS], F16, tag='oa', name='oa')
            nc.scalar.activation(oa[:, 0:w], ps, AF.Identity, bias=cv['bo'][j])
            xsl = xall_c[ch][sb][j]
            nc.vector.tensor_add(rz[j][:, off:off + w], oa[:, 0:w],
                                 xsl[:, off - sb * BS:off - sb * BS + w])

        U2 = [ap.tile([128, C + 1], F16, tag=f'u2_{i}', name=f'u2{i}_{ch}')
              for i in range(NT)]
        for i in range(NT):
            nc.vector.tensor_copy(U2[i][:, 0:1], U2col[i][:])
        fki = [None, None]
        gemm('wo', wkvrs, o_cb, ranges=RO, sbs=(0,))
        fki[0] = ln_sb(rzs[0], U2, 0, mix=('fmixk', 'fk', RF))
        gemm('wo', wkvrs, o_cb, ranges=RO, sbs=(1,))
        fki[1] = ln_sb(rzs[1], U2, 1, mix=('fmixk', 'fk', RF))
        for i in range(NT):
            nc.vector.tensor_copy(U2col[i][:], U2[i][:, C:C + 1])

        # ---- ffnk GEMM -> kf2 = relu(kf)^2 (reuses ek slots) ----
        kf2 = [ap.tile([128, C], F16, tag=f'ek{i}', name=f'kf{i}_{ch}')
               for i in range(NT)]

        def fk_cb(j, sb, ps, off, w):
            kf = scr.tile([128, BS], F16, tag='kf', name='kf')
            nc.scalar.activation(kf[:, 0:w], ps, AF.Relu, bias=cv['bfk'][j])
            nc.scalar.activation(kf2[j][:, off:off + w], kf[:, 0:w], AF.Square)
        gemm('wfk', fki, fk_cb, ranges=RF, sbs=(0,), split_first=True)
        gemm('wfk', fki, fk_cb, ranges=RF, sbs=(1,))

        # ---- ffnr GEMM -> rf (reuses ekv slots) ----
        fri = mk_mix(U2, 'fmixr', 'fr', RF)

        # next chunk's front half, emitted under the FFN GEMMs. Must come
        # after fri (U2's last reader) so U(ch+1)'s writes — same tile slot —
        # never wait on DVE work queued behind them.
        if ch + 1 < NCH:
            front = att_front(ch + 1)
        rf = [ap.tile([128, C], F16, tag=f'ekv{i}', name=f'rf{i}_{ch}')
              for i in range(NT)]
        gemm('wfr', fri,
             lambda j, sb, ps, off, w: nc.scalar.activation(
                 rf[j][:, off:off + w], ps, AF.Sigmoid, bias=cv['bfr'][j]),
             ranges=RF)

        # ---- ffnv GEMM -> out = (kf2 @ Wfv + bfv) * rf + rz ----
        kf2s = [[kf2[i][:, off:off + w] for i in range(NT)]
                for sb, (off, w) in enumerate(RF)]

        def fv_cb(j, sb, ps, off, w):
            t3 = scr.tile([128, BS], F16, tag='t3', name='t3')
            nc.scalar.activation(t3[:, 0:w], ps, AF.Identity, bias=cv['bfv'][j])
            t4 = scr.tile([128, BS], F16, tag='t4', name='t4')
            nc.vector.tensor_mul(t4[:, 0:w], t3[:, 0:w], rf[j][:, off:off + w])
            ot = scr.tile([128, BS], F16, tag='ot', name='ot')
            nc.vector.tensor_add(ot[:, 0:w], t4[:, 0:w], rz[j][:, off:off + w])
            t0 = ch * C + off
            nc.sync.dma_start(outTt[j, :, t0:t0 + w], ot[:, 0:w])
        gemm('wfv', kf2s, fv_cb, ranges=RF)


def prep_inputs(inputs):
    f32, f16 = np.float32, np.float16
    x = np.asarray(inputs['x'], f32)
    g1, b1 = np.asarray(inputs['ln1_g'], f32), np.asarray(inputs['ln1_b'], f32)
    g2, b2 = np.asarray(inputs['ln2_g'], f32), np.asarray(inputs['ln2_b'], f32)
    W, Bv = {}, {}
    for key, nm, g, b in [('wk', 'attk', g1, b1), ('wv', 'attv', g1, b1),
                          ('wr', 'attr', g1, b1), ('wfk', 'ffnk', g2, b2),
                          ('wfr', 'ffnr', g2, b2)]:
        w = np.asarray(inputs[nm + '_w'], f32)
        W[key] = np.ascontiguousarray((w * g[None, :]).T)
        Bv[key] = (np.asarray(inputs[nm + '_b'], f32) + w @ b).astype(f32)
    for key, nm in [('wo', 'atto'), ('wfv', 'ffnv')]:
        w = np.asarray(inputs[nm + '_w'], f32)
        W[key] = np.ascontiguousarray(w.T)
        Bv[key] = np.asarray(inputs[nm + '_b'], f32)
    Wp = {}
    for key, wt in W.items():
        wp = wt.astype(f16).reshape(NT, 128, NJG, JQ * 128)
        Wp[key] = np.ascontiguousarray(
            wp.transpose(2, 1, 0, 3).reshape(NJG, 128, NT * JQ * 128))
    bmap = dict(zip(BNAMES, ['wk', 'wv', 'wr', 'wo', 'wfk', 'wfv', 'wfr']))
    mixes = {'mixk': inputs['attmixk'], 'mixv': inputs['attmixv'],
             'mixr': inputs['attmixr'], 'fmixk': inputs['ffnmixk'],
             'fmixr': inputs['ffnmixr']}
    ew = np.exp(-np.exp(np.asarray(inputs['time_decay'], f32))).astype(f32)
    eu = np.exp(np.asarray(inputs['time_first'], f32)).astype(f32)
    xt = np.ascontiguousarray(x.T)

    def colmat(vec):
        return np.asarray(vec, f32).reshape(NT, 128).T  # [128, NT]

    in_maps = []
    for c in range(NCORES):
        s = c * TLOC
        idx = (np.arange(s - H, s + TLOC)) % T
        xc = xt[:, idx].astype(f16)                      # [D, TBUF]
        xp = xc.reshape(NT, 128, 2 * NCH, BS)
        m = {'xP': np.ascontiguousarray(
            xp.transpose(2, 1, 0, 3).reshape(2 * NCH, 128, NT * BS))}
        for k in WNAMES:
            m[k] = Wp[k]
        cvcols = {}
        for k in BNAMES:
            cvcols[k] = colmat(Bv[bmap[k]])
        for k, v in mixes.items():
            cvcols[k] = colmat(v)
        cvcols['ew'] = colmat(ew)
        cvcols['eu'] = colmat(eu)
        cvcols['cmask'] = np.full((128, NT), 0.0 if c == 0 else 1.0, f32)
        m['cvP'] = np.ascontiguousarray(
            np.concatenate([cvcols[n] for n in CVNAMES], axis=1))
        m['onescol'] = np.ones((128, 1), f16)
        m['onesrow'] = np.ones((1, 128), f32)
        in_maps.append(m)
    return in_maps


_CACHED = {}
TRACE = False
LAST = {}


def kernel(**inputs):
    if 'nc' not in _CACHED:
        _CACHED['nc'] = build_kernel()
    nc = _CACHED['nc']
    in_maps = prep_inputs(inputs)
    kw = {}
    if TRACE:
        kw = dict(trace=True, trace_cores=list(range(NCORES)))
    res = run_bass_kernel_spmd(nc, in_maps, list(range(NCORES)), **kw)
    LAST['res'] = res
    parts = []
    for c in range(NCORES):
        oc = np.asarray(res.results[c]['outT'])         # [D, TBUF] f16
        parts.append(oc[:, H:].T.astype(np.float32))
    return np.ascontiguousarray(np.concatenate(parts, axis=0))


if __name__ == '__main__':
    import reference
    inputs = {k: np.asarray(v) for k, v in reference.setup_inputs().items()}
    out = kernel(**inputs)
    print('out', out.shape, out.dtype)
